# revision 1
# baseline (speedup 1.0000x reference)
"""Deformable conv (3x3, with offset-predicting conv) for Trainium2, 8 cores.

Sharding: pure data parallel. Core k handles sample b = k//2, output row block
(k%2)*48 .. +48 (48 rows x 96 cols = 4608 pixels). Full sample's x is available
to every core as a DRAM token table, so gathers are purely local.

Per-core pipeline (all on one NeuronCore, scheduled by Tile):
  A. offset conv (3x3, C=256 -> 18) as 18 PE matmuls per 4-row chunk
  B. PE-transpose offsets into pixel-partition layout [128px, tile, 18]
  C. DVE index/weight math: bilinear corner weights (zero-pad semantics exactly
     like the reference) + int16 gather token indices
  D. fold indices into the SWDGE "wrapped 16-partition" layout + replicate x8
  E. per (stage of 512 px, tap): dma_gather of (x0,x0+1) bf16 token pairs for
     corner rows y0,y0+1 -> [128px, (tile,a), 512c]; DVE 4-term FMA combine with
     per-partition (per-pixel) scalar weights; PE transpose to channel layout;
     PE matmul accumulating over (c,tap) into PSUM [o,px]; bias + store.
"""

import dataclasses

import numpy as np

import concourse.bacc as bacc
import concourse.bass as bass
import concourse.mybir as mybir
import concourse.tile as tile
from concourse import bass_utils, masks
from concourse.mybir import ActivationFunctionType as Act
from concourse.mybir import AluOpType as Op

P = 128
B, C, H, W, O = 4, 256, 96, 96, 256
K = 3
K2 = 9
NCORES = 8
ROWS = 48                      # output rows per core
NPIX = ROWS * W                # 4608
NTILE = NPIX // P              # 36 pixel tiles of 128
NSTAGE = 9                     # stages of 512 px
TPS = 4                        # pixel tiles per stage
SPX = TPS * P                  # 512
PADH, PADW = ROWS + 2, W + 2   # 50, 98
NTOK = H * W                   # 9216
CONV_ROWS_PER_CHUNK = 4        # offset-conv N chunk = 4 rows = 384 cols
NCHUNK = ROWS // CONV_ROWS_PER_CHUNK  # 12
BF = mybir.dt.bfloat16
F32 = mybir.dt.float32
I16 = mybir.dt.int16

_BUILT = {}


def _emit(tc, nc, io):
    xt, xc, wofl, boff, wdcl, bdc, pyb, pxb, out = io

    with (
        tc.tile_pool(name="const", bufs=1) as cpool,
        tc.tile_pool(name="sbig", bufs=1) as spool,
    ):
        ident_bf = cpool.tile([P, P], BF, tag="idbf", name="idbf")
        ident_f = cpool.tile([P, P], F32, tag="idf", name="idf")
        masks.make_identity(nc, ident_bf[:])
        masks.make_identity(nc, ident_f[:])

        # ---- persistent SBUF buffers ----
        xc_sb = spool.tile([P, 2, PADH * PADW], BF, tag="xc", name="xc")      # 19.6KB/part
        wofl_sb = spool.tile([P, 2, K2, 18], BF, tag="wofl", name="wofl")
        wdcl_sb = spool.tile([P, K2, 2, 2, P], BF, tag="wdcl", name="wdcl")     # 9.2KB/part
        boff_sb = spool.tile([18, 1], F32, tag="boff", name="boff")
        bdc_sb = spool.tile([P, 2], F32, tag="bdc", name="bdc")
        pyb_sb = spool.tile([P, NTILE, K2], F32, tag="pyb", name="pyb")
        pxb_sb = spool.tile([P, NTILE, K2], F32, tag="pxb", name="pxb")
        off_sb = spool.tile([18, NPIX], F32, tag="off", name="off")            # 18 parts
        doff = spool.tile([P, NTILE, 18], F32, tag="doff", name="doff")
        wt = spool.tile([P, NTILE, K2, 4], F32, tag="wt", name="wt")          # corner wgts
        cidx = spool.tile([P, K2, NTILE, 2], I16, tag="cidx", name="cidx")      # f=(tap,t,a)
        twrap = spool.tile([P, K2 * NTILE * 2 * 8], I16, tag="twrap", name="twrap")  # 10.4KB

        nc.sync.dma_start(xc_sb[:], xc)
        nc.sync.dma_start(wofl_sb[:], wofl)
        nc.sync.dma_start(wdcl_sb[:], wdcl)
        nc.sync.dma_start(boff_sb[:], boff)
        nc.sync.dma_start(bdc_sb[:], bdc)
        nc.sync.dma_start(pyb_sb[:], pyb)
        nc.sync.dma_start(pxb_sb[:], pxb)

        # ---- A: offset conv ----
        with tc.tile_pool(name="psA", bufs=2, space="PSUM") as psa:
            for ch_i in range(NCHUNK):
                ncols = CONV_ROWS_PER_CHUNK * W  # 384
                ps = psa.tile([18, ncols], F32, tag="psoff", name="psoff")
                n_mm = 2 * K2
                mm = 0
                xcf = xc_sb[:]
                for chalf in range(2):
                    for tap in range(K2):
                        ti, tj = tap // K, tap % K
                        rhs = dataclasses.replace(
                            xcf,
                            ap=[
                                [xcf.ap[0][0], P],
                                [PADW, CONV_ROWS_PER_CHUNK],
                                [1, W],
                            ],
                            offset=xcf.offset
                            + chalf * (PADH * PADW)
                            + ((ch_i * CONV_ROWS_PER_CHUNK + ti) * PADW + tj),
                        )
                        nc.tensor.matmul(
                            ps[:],
                            wofl_sb[:, chalf, tap],
                            rhs,
                            start=(mm == 0),
                            stop=(mm == n_mm - 1),
                        )
                        mm += 1
                nc.scalar.activation(
                    off_sb[:, ch_i * ncols : (ch_i + 1) * ncols],
                    ps[:],
                    Act.Identity,
                    bias=boff_sb[:],
                )

        # ---- B: transpose offsets to pixel layout ----
        with tc.tile_pool(name="psB", bufs=4, space="PSUM") as psb:
            for t in range(NTILE):
                pt = psb.tile([P, 18], F32, tag="pofft", name="pofft")
                nc.tensor.transpose(
                    pt[:], off_sb[:, t * P : (t + 1) * P], ident_f[:18, :18]
                )
                nc.scalar.copy(doff[:, t, :], pt[:])

        # ---- C: index / weight math (DVE over [128, 36*9]) ----
        with tc.tile_pool(name="scr", bufs=1) as scr:
            sh = [P, NTILE, K2]

            def tmp(tag):
                return scr.tile(sh, F32, tag=tag, name=tag)

            # py16/px16 = sample coords + 16 (strictly positive); y0/x0 here
            # are floor(py)+16 etc. All downstream constants are shifted +16.
            MAGIC = 8388608.0  # 2^23
            dy = doff[:, :, 0:18:2]
            dx = doff[:, :, 1:18:2]
            py = tmp("py")
            px = tmp("px")
            nc.vector.tensor_tensor(py[:], pyb_sb[:], dy, Op.add)
            nc.vector.tensor_tensor(px[:], pxb_sb[:], dx, Op.add)
            nc.vector.tensor_scalar(py[:], py[:], 16.0, None, Op.add)
            nc.vector.tensor_scalar(px[:], px[:], 16.0, None, Op.add)
            y0 = tmp("y0")
            x0 = tmp("x0")
            nc.vector.tensor_scalar(y0[:], py[:], -0.4999999, None, Op.add)
            nc.vector.tensor_scalar(y0[:], y0[:], MAGIC, -MAGIC, Op.add, Op.add)
            nc.vector.tensor_scalar(x0[:], px[:], -0.4999999, None, Op.add)
            nc.vector.tensor_scalar(x0[:], x0[:], MAGIC, -MAGIC, Op.add, Op.add)
            ly = tmp("ly")
            lx = tmp("lx")
            nc.vector.tensor_tensor(ly[:], py[:], y0[:], Op.subtract)
            nc.vector.tensor_tensor(lx[:], px[:], x0[:], Op.subtract)

            ta_ = tmp("ta")
            tb_ = tmp("tb")
            tc_ = tmp("tc")
            td_ = tmp("td")
            # y weights: wy0 = (1-ly)*[0<=y0<=95], wy1 = ly*[0<=y0+1<=95]
            # (all bounds shifted +16)
            nc.vector.tensor_scalar(ta_[:], y0[:], 16.0, None, Op.is_ge)
            nc.vector.tensor_scalar(tb_[:], y0[:], 111.0, None, Op.is_le)
            vy0 = tmp("vy0")
            nc.vector.tensor_tensor(vy0[:], ta_[:], tb_[:], Op.mult)
            nc.vector.tensor_scalar(ta_[:], y0[:], 15.0, None, Op.is_ge)
            nc.vector.tensor_scalar(tb_[:], y0[:], 110.0, None, Op.is_le)
            vy1 = tmp("vy1")
            nc.vector.tensor_tensor(vy1[:], ta_[:], tb_[:], Op.mult)
            wy0 = tmp("wy0")
            wy1 = tmp("wy1")
            nc.vector.tensor_scalar(tc_[:], ly[:], -1.0, 1.0, Op.mult, Op.add)
            nc.vector.tensor_tensor(wy0[:], tc_[:], vy0[:], Op.mult)
            nc.vector.tensor_tensor(wy1[:], ly[:], vy1[:], Op.mult)

            # x pair weights on tokens (xb, xb+1), xb = clip(x0,0,95):
            # wA = (1-lx)*[0<=x0<=95] + lx*[x0==-1] ; wB = lx*[0<=x0<=94]
            # (all bounds shifted +16)
            nc.vector.tensor_scalar(ta_[:], x0[:], 16.0, None, Op.is_ge)
            nc.vector.tensor_scalar(tb_[:], x0[:], 111.0, None, Op.is_le)
            vx = tmp("vx")
            nc.vector.tensor_tensor(vx[:], ta_[:], tb_[:], Op.mult)
            nc.vector.tensor_scalar(tb_[:], x0[:], 110.0, None, Op.is_le)
            vxb = tmp("vxb")
            nc.vector.tensor_tensor(vxb[:], ta_[:], tb_[:], Op.mult)
            nc.vector.tensor_scalar(td_[:], x0[:], 15.0, None, Op.is_equal)
            wa = tmp("wa")
            wb = tmp("wb")
            nc.vector.tensor_scalar(tc_[:], lx[:], -1.0, 1.0, Op.mult, Op.add)
            nc.vector.tensor_tensor(tc_[:], tc_[:], vx[:], Op.mult)
            nc.vector.tensor_tensor(td_[:], lx[:], td_[:], Op.mult)
            nc.vector.tensor_tensor(wa[:], tc_[:], td_[:], Op.add)
            nc.vector.tensor_tensor(wb[:], lx[:], vxb[:], Op.mult)

            # final 4 corner weights -> wt[:, t, tap, 0..3] = A0,B0,A1,B1
            nc.vector.tensor_tensor(wt[:, :, :, 0], wy0[:], wa[:], Op.mult)
            nc.vector.tensor_tensor(wt[:, :, :, 1], wy0[:], wb[:], Op.mult)
            nc.vector.tensor_tensor(wt[:, :, :, 2], wy1[:], wa[:], Op.mult)
            nc.vector.tensor_tensor(wt[:, :, :, 3], wy1[:], wb[:], Op.mult)

            # gather base indices: idx_a = clip(y0+a,0,95)*96 + clip(x0,0,95)
            # (y0/x0 carry +16: un-shift via the final constant)
            xb = tmp("xb")
            nc.vector.tensor_scalar(xb[:], x0[:], 16.0, 111.0, Op.max, Op.min)
            yb0 = tmp("yb0")
            nc.vector.tensor_scalar(yb0[:], y0[:], 16.0, 111.0, Op.max, Op.min)
            yb1 = tmp("yb1")
            nc.vector.tensor_scalar(yb1[:], y0[:], 15.0, 110.0, Op.max, Op.min)
            idx0 = tmp("idx0")
            idx1 = tmp("idx1")
            nc.vector.scalar_tensor_tensor(
                idx0[:], yb0[:], 96.0, xb[:], Op.mult, Op.add
            )
            nc.vector.tensor_scalar(idx0[:], idx0[:], -1552.0, None, Op.add)
            nc.vector.scalar_tensor_tensor(
                idx1[:], yb1[:], 96.0, xb[:], Op.mult, Op.add
            )
            nc.vector.tensor_scalar(idx1[:], idx1[:], -1456.0, None, Op.add)
            # cidx[p, tap, t, a] int16
            nc.vector.tensor_copy(cidx[:, :, :, 0].transpose((0, 2, 1)), idx0[:])
            nc.vector.tensor_copy(cidx[:, :, :, 1].transpose((0, 2, 1)), idx1[:])

        # ---- D: fold cidx into wrapped layout twrap[16g+r, f*8+k] ----
        # cidx free f = tap*72 + t*2 + a (int16 elems); value for pixel
        # p = t*128 + 16k + r lives at partition 16k+r.
        FD = K2 * NTILE * 2  # 648
        cflat = cidx[:]
        pitch_c = cflat.ap[0][0]
        tw = twrap[:]
        pitch_t = tw.ap[0][0]
        for r in range(16):
            for kk in range(8):
                src = dataclasses.replace(
                    cflat,
                    ap=[[pitch_c, 1], [1, FD]],
                    offset=cflat.offset + (16 * kk + r) * pitch_c,
                )
                dst = dataclasses.replace(
                    tw,
                    ap=[[pitch_t, 1], [8, FD]],
                    offset=tw.offset + r * pitch_t + kk,
                )
                nc.sync.dma_start(dst, src)
        for g in range(1, 8):
            nc.sync.dma_start(twrap[16 * g : 16 * (g + 1), :], twrap[0:16, :])

        # ---- E: main loop ----
        with (
            tc.tile_pool(name="gpool", bufs=3) as gpool,
            tc.tile_pool(name="vpool", bufs=4) as vpool,
            tc.tile_pool(name="rpool", bufs=3) as rpool,
            tc.tile_pool(name="opool", bufs=3) as opool,
            tc.tile_pool(name="psT", bufs=4, space="PSUM") as pst,
            tc.tile_pool(name="psO", bufs=2, space="PSUM") as pso,
        ):
            # overlapped-window view of the token table: [NTOK, 512] stride 256
            xt_ap = xt
            xt_win = dataclasses.replace(
                xt_ap, ap=[[C, NTOK], [1, 2 * C]], offset=0
            )
            for s in range(NSTAGE):
                po = [pso.tile([P, SPX], F32, tag=f"po{oh}", name=f"po{oh}") for oh in range(2)]
                for tap in range(K2):
                    g = gpool.tile([P, 2 * TPS, 2 * C], BF, tag="g", name="g")
                    idxs = twrap[:, (tap * NTILE * 2 + s * TPS * 2) * 8
                                 : (tap * NTILE * 2 + (s + 1) * TPS * 2) * 8]
                    nc.gpsimd.dma_gather(
                        g[:],
                        xt_win,
                        idxs,
                        2 * SPX,
                        2 * SPX,
                        elem_size=2 * C,
                        elem_step=C,
                    )
                    rst = [rpool.tile([P, SPX], BF, tag=f"r{c}", name=f"r{c}") for c in range(2)]
                    for t in range(TPS):
                        v = vpool.tile([P, C], BF, tag="v", name="v")
                        gt = t * 2
                        wcol = wt[:, s * TPS + t, tap, :]
                        nc.vector.tensor_scalar(
                            v[:], g[:, gt, 0:C], wcol[:, 0:1], None, Op.mult
                        )
                        nc.vector.scalar_tensor_tensor(
                            v[:], g[:, gt, C : 2 * C], wcol[:, 1:2], v[:],
                            Op.mult, Op.add,
                        )
                        nc.vector.scalar_tensor_tensor(
                            v[:], g[:, gt + 1, 0:C], wcol[:, 2:3], v[:],
                            Op.mult, Op.add,
                        )
                        nc.vector.scalar_tensor_tensor(
                            v[:], g[:, gt + 1, C : 2 * C], wcol[:, 3:4], v[:],
                            Op.mult, Op.add,
                        )
                        for chalf in range(2):
                            ptr = pst.tile([P, P], BF, tag="ptr", name="ptr")
                            nc.tensor.transpose(
                                ptr[:],
                                v[:, chalf * P : (chalf + 1) * P],
                                ident_bf[:],
                            )
                            nc.scalar.copy(
                                rst[chalf][:, t * P : (t + 1) * P], ptr[:]
                            )
                    for chalf in range(2):
                        for oh in range(2):
                            nc.tensor.matmul(
                                po[oh][:],
                                wdcl_sb[:, tap, chalf, oh],
                                rst[chalf][:],
                                start=(tap == 0 and chalf == 0),
                                stop=(tap == K2 - 1 and chalf == 1),
                            )
                for oh in range(2):
                    ob = opool.tile([P, SPX], F32, tag="ob", name="ob")
                    nc.scalar.activation(
                        ob[:], po[oh][:], Act.Identity, bias=bdc_sb[:, oh : oh + 1]
                    )
                    nc.sync.dma_start(
                        out[oh, :, s * SPX : (s + 1) * SPX], ob[:]
                    )


def _build():
    if "nc" in _BUILT:
        return _BUILT["nc"]
    nc = bacc.Bacc(
        "TRN2",
        target_bir_lowering=False,
        debug=False,
        enable_asserts=False,
        num_devices=NCORES,
    )
    xt = nc.dram_tensor("xt", [NTOK + 1, C], BF, kind="ExternalInput").ap()
    xc = nc.dram_tensor("xc", [P, 2, PADH * PADW], BF, kind="ExternalInput").ap()
    wofl = nc.dram_tensor("wofl", [P, 2, K2, 18], BF, kind="ExternalInput").ap()
    boff = nc.dram_tensor("boff", [18, 1], F32, kind="ExternalInput").ap()
    wdcl = nc.dram_tensor("wdcl", [P, K2, 2, 2, P], BF, kind="ExternalInput").ap()
    bdc = nc.dram_tensor("bdc", [P, 2], F32, kind="ExternalInput").ap()
    pyb = nc.dram_tensor("pyb", [P, NTILE, K2], F32, kind="ExternalInput").ap()
    pxb = nc.dram_tensor("pxb", [P, NTILE, K2], F32, kind="ExternalInput").ap()
    out = nc.dram_tensor("out", [2, P, NPIX], F32, kind="ExternalOutput").ap()
    with tile.TileContext(nc) as tc:
        _emit(tc, nc, (xt, xc, wofl, boff, wdcl, bdc, pyb, pxb, out))
    nc.compile()
    _BUILT["nc"] = nc
    return nc


def _prep_core(k, x, w_off, b_off, w_dc, b_dc):
    b, half = k // 2, k % 2
    y0 = half * ROWS
    xs = x[b]  # [C,H,W] f32
    xt = np.zeros((NTOK + 1, C), np.float32)
    xt[:NTOK] = xs.transpose(1, 2, 0).reshape(NTOK, C)
    xc = np.zeros((C, PADH, PADW), np.float32)
    r0, r1 = max(0, y0 - 1), min(H, y0 + ROWS + 1)
    xc[:, (r0 - (y0 - 1)) : (r1 - (y0 - 1)), 1 : 1 + W] = xs[:, r0:r1, :]
    xc = xc.reshape(2, P, PADH * PADW).transpose(1, 0, 2)

    wofl = (
        w_off.reshape(2 * K2, 2, P, K2)   # [oc, chalf, c, tap]
        .transpose(2, 1, 3, 0)            # [c, chalf, tap, oc]
        .copy()
    )
    wdcl = (
        w_dc.reshape(2, P, 2, P, K2)      # [oh, o, chalf, c, tap]
        .transpose(3, 4, 2, 0, 1)         # [c, tap, chalf, oh, o]
        .copy()
    )
    bdc = b_dc.reshape(2, P).transpose(1, 0).copy()

    pp = np.arange(NPIX)
    yg = y0 + pp // W
    xg = pp % W
    ti = (np.arange(K2) // K)[None, :]
    tj = (np.arange(K2) % K)[None, :]
    pyb = (yg[:, None] - 1 + ti).astype(np.float32).reshape(NTILE, P, K2)
    pxb = (xg[:, None] - 1 + tj).astype(np.float32).reshape(NTILE, P, K2)

    import ml_dtypes

    bf16 = ml_dtypes.bfloat16
    return {
        "xt": xt.astype(bf16),
        "xc": xc.astype(bf16),
        "wofl": wofl.astype(bf16),
        "boff": b_off.reshape(18, 1).astype(np.float32),
        "wdcl": wdcl.astype(bf16),
        "bdc": bdc.astype(np.float32),
        "pyb": pyb.transpose(1, 0, 2).copy(),
        "pxb": pxb.transpose(1, 0, 2).copy(),
    }


def kernel(x, w_off, b_off, w_dc, b_dc, _trace=False):
    nc = _build()
    x = np.asarray(x, np.float32)
    w_off = np.asarray(w_off, np.float32)
    b_off = np.asarray(b_off, np.float32)
    w_dc = np.asarray(w_dc, np.float32)
    b_dc = np.asarray(b_dc, np.float32)
    in_maps = [
        _prep_core(k, x, w_off, b_off, w_dc, b_dc) for k in range(NCORES)
    ]
    res = bass_utils.run_bass_kernel_spmd(
        nc, in_maps, core_ids=list(range(NCORES)), trace=_trace
    )
    out = np.empty((B, O, H, W), np.float32)
    for k in range(NCORES):
        b, half = k // 2, k % 2
        o = res.results[k]["out"]  # [2,128,4608]
        out[b, :, half * ROWS : (half + 1) * ROWS, :] = o.reshape(
            O, ROWS, W
        )
    if _trace:
        return out, res
    return out



# revision 11
# speedup vs baseline: 1.6390x; 1.6390x over previous
"""Deformable conv (3x3, with offset-predicting conv) for Trainium2, 8 cores.

Sharding: pure data parallel. Core k handles sample b = k//2, output row block
(k%2)*48 .. +48 (48 rows x 96 cols = 4608 pixels). Full sample's x is available
to every core as a DRAM token table, so gathers are purely local.

Per-core pipeline (all on one NeuronCore, scheduled by Tile):
  A. offset conv (3x3, C=256 -> 18) as 18 PE matmuls per 4-row chunk
  B. PE-transpose offsets into pixel-partition layout [128px, tile, 18]
  C. DVE index/weight math: bilinear corner weights (zero-pad semantics exactly
     like the reference) + int16 gather token indices
  D. fold indices into the SWDGE "wrapped 16-partition" layout + replicate x8
  E. per (stage of 512 px, tap): dma_gather from a host-built 2x2-patch table
     U[t] = (x[t-96], x[t-95], x[t], x[t+1]) -- ONE 2KB descriptor per
     (pixel, tap) fetching all 4 bilinear corners -> [128px, tile, 4*256];
     DVE 4-term FMA combine with per-partition (per-pixel) scalar weights;
     PE transpose to channel layout; PE matmul accumulating over (c,tap)
     into PSUM [o,px]; bias + store.
"""

import dataclasses

import numpy as np

import concourse.bacc as bacc
import concourse.bass as bass
import concourse.mybir as mybir
import concourse.tile as tile
from concourse import bass_utils, masks
from concourse.mybir import ActivationFunctionType as Act
from concourse.mybir import AluOpType as Op

P = 128
B, C, H, W, O = 4, 256, 96, 96, 256
K = 3
K2 = 9
NCORES = 8
ROWS = 48                      # output rows per core
NPIX = ROWS * W                # 4608
NTILE = NPIX // P              # 36 pixel tiles of 128
NSTAGE = 9                     # stages of 512 px
TPS = 4                        # pixel tiles per stage
SPX = TPS * P                  # 512
PADH, PADW = ROWS + 2, W + 2   # 50, 98
NTOK = H * W                   # 9216
NGTOK = NTOK + W               # 9312 patch-table entries (row base -1..96)
CONV_ROWS_PER_CHUNK = 4        # offset-conv N chunk = 4 rows = 384 cols
NCHUNK = ROWS // CONV_ROWS_PER_CHUNK  # 12
BF = mybir.dt.bfloat16
F32 = mybir.dt.float32
I16 = mybir.dt.int16

_BUILT = {}


def _emit(tc, nc, io):
    xt, xc, wofl, boff, wdcl, bdc, pyb, pxb, out = io

    with (
        tc.tile_pool(name="const", bufs=1) as cpool,
        tc.tile_pool(name="sbig", bufs=1) as spool,
    ):
        ident_bf = cpool.tile([P, P], BF, tag="idbf", name="idbf")
        ident_f = cpool.tile([P, P], F32, tag="idf", name="idf")
        masks.make_identity(nc, ident_bf[:])
        masks.make_identity(nc, ident_f[:])

        # ---- persistent SBUF buffers ----
        xc_sb = spool.tile([P, 2, PADH * PADW], BF, tag="xc", name="xc")      # 19.6KB/part
        wofl_sb = spool.tile([P, 2, K2, 18], BF, tag="wofl", name="wofl")
        wdcl_sb = spool.tile([P, K2, 2, 2, P], BF, tag="wdcl", name="wdcl")     # 9.2KB/part
        boff_sb = spool.tile([18, 1], F32, tag="boff", name="boff")
        bdc_sb = spool.tile([P, 2], F32, tag="bdc", name="bdc")
        pyb_sb = spool.tile([P, NTILE, K2], F32, tag="pyb", name="pyb")
        pxb_sb = spool.tile([P, NTILE, K2], F32, tag="pxb", name="pxb")
        off_sb = spool.tile([18, NPIX], F32, tag="off", name="off")            # 18 parts
        doff = spool.tile([P, NTILE, 18], F32, tag="doff", name="doff")
        wt = spool.tile([P, NTILE, K2, 4], F32, tag="wt", name="wt")          # corner wgts
        cidx = spool.tile([P, K2, NTILE], I16, tag="cidx", name="cidx")         # f=(tap,t)
        twrap = spool.tile([P, K2 * NTILE * 8], I16, tag="twrap", name="twrap")  # 5.2KB

        nc.sync.dma_start(xc_sb[:], xc)
        nc.sync.dma_start(wofl_sb[:], wofl)
        nc.sync.dma_start(wdcl_sb[:], wdcl)
        nc.sync.dma_start(boff_sb[:], boff)
        nc.sync.dma_start(bdc_sb[:], bdc)
        nc.sync.dma_start(pyb_sb[:], pyb)
        nc.sync.dma_start(pxb_sb[:], pxb)

        # ---- A: offset conv ----
        with tc.tile_pool(name="psA", bufs=2, space="PSUM") as psa:
            for ch_i in range(NCHUNK):
                ncols = CONV_ROWS_PER_CHUNK * W  # 384
                ps = psa.tile([18, ncols], F32, tag="psoff", name="psoff")
                n_mm = 2 * K2
                mm = 0
                xcf = xc_sb[:]
                for chalf in range(2):
                    for tap in range(K2):
                        ti, tj = tap // K, tap % K
                        rhs = dataclasses.replace(
                            xcf,
                            ap=[
                                [xcf.ap[0][0], P],
                                [PADW, CONV_ROWS_PER_CHUNK],
                                [1, W],
                            ],
                            offset=xcf.offset
                            + chalf * (PADH * PADW)
                            + ((ch_i * CONV_ROWS_PER_CHUNK + ti) * PADW + tj),
                        )
                        nc.tensor.matmul(
                            ps[:],
                            wofl_sb[:, chalf, tap],
                            rhs,
                            start=(mm == 0),
                            stop=(mm == n_mm - 1),
                        )
                        mm += 1
                nc.scalar.activation(
                    off_sb[:, ch_i * ncols : (ch_i + 1) * ncols],
                    ps[:],
                    Act.Identity,
                    bias=boff_sb[:],
                )

        # ---- B: transpose offsets to pixel layout ----
        with tc.tile_pool(name="psB", bufs=4, space="PSUM") as psb:
            for t in range(NTILE):
                pt = psb.tile([P, 18], F32, tag="pofft", name="pofft")
                nc.tensor.transpose(
                    pt[:], off_sb[:, t * P : (t + 1) * P], ident_f[:18, :18]
                )
                nc.scalar.copy(doff[:, t, :], pt[:])

        # ---- C: index / weight math (DVE over [128, 36*9]) ----
        with tc.tile_pool(name="scr", bufs=1) as scr:
            sh = [P, NTILE, K2]

            def tmp(tag):
                return scr.tile(sh, F32, tag=tag, name=tag)

            # py16/px16 = sample coords + 16 (strictly positive); y0/x0 here
            # are floor(py)+16 etc. All downstream constants are shifted +16.
            MAGIC = 8388608.0  # 2^23
            dy = doff[:, :, 0:18:2]
            dx = doff[:, :, 1:18:2]
            py = tmp("py")
            px = tmp("px")
            nc.vector.tensor_tensor(py[:], pyb_sb[:], dy, Op.add)
            nc.vector.tensor_tensor(px[:], pxb_sb[:], dx, Op.add)
            nc.vector.tensor_scalar(py[:], py[:], 16.0, None, Op.add)
            nc.vector.tensor_scalar(px[:], px[:], 16.0, None, Op.add)
            y0 = tmp("y0")
            x0 = tmp("x0")
            nc.vector.tensor_scalar(y0[:], py[:], -0.4999999, None, Op.add)
            nc.vector.tensor_scalar(y0[:], y0[:], MAGIC, -MAGIC, Op.add, Op.add)
            nc.vector.tensor_scalar(x0[:], px[:], -0.4999999, None, Op.add)
            nc.vector.tensor_scalar(x0[:], x0[:], MAGIC, -MAGIC, Op.add, Op.add)
            ly = tmp("ly")
            lx = tmp("lx")
            nc.vector.tensor_tensor(ly[:], py[:], y0[:], Op.subtract)
            nc.vector.tensor_tensor(lx[:], px[:], x0[:], Op.subtract)

            ta_ = tmp("ta")
            tb_ = tmp("tb")
            tc_ = tmp("tc")
            td_ = tmp("td")
            # y weights: wy0 = (1-ly)*[0<=y0<=95], wy1 = ly*[0<=y0+1<=95]
            # (all bounds shifted +16)
            nc.vector.tensor_scalar(ta_[:], y0[:], 16.0, None, Op.is_ge)
            nc.vector.tensor_scalar(tb_[:], y0[:], 111.0, None, Op.is_le)
            vy0 = tmp("vy0")
            nc.vector.tensor_tensor(vy0[:], ta_[:], tb_[:], Op.mult)
            nc.vector.tensor_scalar(ta_[:], y0[:], 15.0, None, Op.is_ge)
            nc.vector.tensor_scalar(tb_[:], y0[:], 110.0, None, Op.is_le)
            vy1 = tmp("vy1")
            nc.vector.tensor_tensor(vy1[:], ta_[:], tb_[:], Op.mult)
            wy0 = tmp("wy0")
            wy1 = tmp("wy1")
            nc.vector.tensor_scalar(tc_[:], ly[:], -1.0, 1.0, Op.mult, Op.add)
            nc.vector.tensor_tensor(wy0[:], tc_[:], vy0[:], Op.mult)
            nc.vector.tensor_tensor(wy1[:], ly[:], vy1[:], Op.mult)

            # x pair weights on tokens (xb, xb+1), xb = clip(x0,0,95):
            # wA = (1-lx)*[0<=x0<=95] + lx*[x0==-1] ; wB = lx*[0<=x0<=94]
            # (all bounds shifted +16)
            nc.vector.tensor_scalar(ta_[:], x0[:], 16.0, None, Op.is_ge)
            nc.vector.tensor_scalar(tb_[:], x0[:], 111.0, None, Op.is_le)
            vx = tmp("vx")
            nc.vector.tensor_tensor(vx[:], ta_[:], tb_[:], Op.mult)
            nc.vector.tensor_scalar(tb_[:], x0[:], 110.0, None, Op.is_le)
            vxb = tmp("vxb")
            nc.vector.tensor_tensor(vxb[:], ta_[:], tb_[:], Op.mult)
            nc.vector.tensor_scalar(td_[:], x0[:], 15.0, None, Op.is_equal)
            wa = tmp("wa")
            wb = tmp("wb")
            nc.vector.tensor_scalar(tc_[:], lx[:], -1.0, 1.0, Op.mult, Op.add)
            nc.vector.tensor_tensor(tc_[:], tc_[:], vx[:], Op.mult)
            nc.vector.tensor_tensor(td_[:], lx[:], td_[:], Op.mult)
            nc.vector.tensor_tensor(wa[:], tc_[:], td_[:], Op.add)
            nc.vector.tensor_tensor(wb[:], lx[:], vxb[:], Op.mult)

            # final 4 corner weights -> wt[:, t, tap, 0..3] = A0,B0,A1,B1
            nc.vector.tensor_tensor(wt[:, :, :, 0], wy0[:], wa[:], Op.mult)
            nc.vector.tensor_tensor(wt[:, :, :, 1], wy0[:], wb[:], Op.mult)
            nc.vector.tensor_tensor(wt[:, :, :, 2], wy1[:], wa[:], Op.mult)
            nc.vector.tensor_tensor(wt[:, :, :, 3], wy1[:], wb[:], Op.mult)

            # patch-table index: idx = clip(y0+1,0,96)*96 + clip(x0,0,95);
            # table entry t holds corners (rows t//96-1, t//96; cols t%96,
            # t%96+1). (y0/x0 carry +16: un-shift via the final constant)
            xb = tmp("xb")
            nc.vector.tensor_scalar(xb[:], x0[:], 16.0, 111.0, Op.max, Op.min)
            ybn = tmp("ybn")
            nc.vector.tensor_scalar(ybn[:], y0[:], 15.0, 111.0, Op.max, Op.min)
            idxn = tmp("idxn")
            nc.vector.scalar_tensor_tensor(
                idxn[:], ybn[:], 96.0, xb[:], Op.mult, Op.add
            )
            nc.vector.tensor_scalar(idxn[:], idxn[:], -1456.0, None, Op.add)
            # cidx[p, tap, t] int16
            nc.vector.tensor_copy(cidx[:].transpose((0, 2, 1)), idxn[:])

        # ---- D: fold cidx into wrapped layout twrap[16g+r, f*8+k] ----
        # cidx free f = tap*36 + t (int16 elems); value for pixel
        # p = t*128 + 16k + r lives at partition 16k+r.
        FD = K2 * NTILE  # 324
        cflat = cidx[:]
        pitch_c = cflat.ap[0][0]
        tw = twrap[:]
        pitch_t = tw.ap[0][0]
        for r in range(16):
            for kk in range(8):
                src = dataclasses.replace(
                    cflat,
                    ap=[[pitch_c, 1], [1, FD]],
                    offset=cflat.offset + (16 * kk + r) * pitch_c,
                )
                dst = dataclasses.replace(
                    tw,
                    ap=[[pitch_t, 1], [8, FD]],
                    offset=tw.offset + r * pitch_t + kk,
                )
                nc.sync.dma_start(dst, src)
        for g in range(1, 8):
            nc.sync.dma_start(twrap[16 * g : 16 * (g + 1), :], twrap[0:16, :])

        # ---- E: main loop ----
        with (
            tc.tile_pool(name="gpool", bufs=4) as gpool,
            tc.tile_pool(name="vpool", bufs=4) as vpool,
            tc.tile_pool(name="rpool", bufs=3) as rpool,
            tc.tile_pool(name="opool", bufs=3) as opool,
            tc.tile_pool(name="psT", bufs=4, space="PSUM") as pst,
            tc.tile_pool(name="psO", bufs=2, space="PSUM") as pso,
        ):
            # 2x2-patch table: entry t = 4 tokens x 256c (2KB)
            xt_ap = xt
            xt_win = dataclasses.replace(
                xt_ap, ap=[[4 * C, NGTOK], [1, 4 * C]], offset=0
            )
            for s in range(NSTAGE):
                po = [pso.tile([P, SPX], F32, tag=f"po{oh}", name=f"po{oh}") for oh in range(2)]
                for tap in range(K2):
                    g = gpool.tile([P, TPS, 4 * C], BF, tag="g", name="g")
                    idxs = twrap[:, (tap * NTILE + s * TPS) * 8
                                 : (tap * NTILE + (s + 1) * TPS) * 8]
                    nc.gpsimd.dma_gather(
                        g[:],
                        xt_win,
                        idxs,
                        SPX,
                        SPX,
                        elem_size=4 * C,
                        elem_step=4 * C,
                    )
                    rst = [rpool.tile([P, SPX], BF, tag=f"r{c}", name=f"r{c}") for c in range(2)]
                    for t in range(TPS):
                        v = vpool.tile([P, C], BF, tag="v", name="v")
                        wcol = wt[:, s * TPS + t, tap, :]
                        nc.vector.tensor_scalar(
                            v[:], g[:, t, 0:C], wcol[:, 0:1], None, Op.mult
                        )
                        nc.vector.scalar_tensor_tensor(
                            v[:], g[:, t, C : 2 * C], wcol[:, 1:2], v[:],
                            Op.mult, Op.add,
                        )
                        nc.vector.scalar_tensor_tensor(
                            v[:], g[:, t, 2 * C : 3 * C], wcol[:, 2:3], v[:],
                            Op.mult, Op.add,
                        )
                        nc.vector.scalar_tensor_tensor(
                            v[:], g[:, t, 3 * C : 4 * C], wcol[:, 3:4], v[:],
                            Op.mult, Op.add,
                        )
                        for chalf in range(2):
                            ptr = pst.tile([P, P], BF, tag="ptr", name="ptr")
                            nc.tensor.transpose(
                                ptr[:],
                                v[:, chalf * P : (chalf + 1) * P],
                                ident_bf[:],
                            )
                            nc.scalar.copy(
                                rst[chalf][:, t * P : (t + 1) * P], ptr[:]
                            )
                    for chalf in range(2):
                        for oh in range(2):
                            nc.tensor.matmul(
                                po[oh][:],
                                wdcl_sb[:, tap, chalf, oh],
                                rst[chalf][:],
                                start=(tap == 0 and chalf == 0),
                                stop=(tap == K2 - 1 and chalf == 1),
                            )
                for oh in range(2):
                    ob = opool.tile([P, SPX], F32, tag="ob", name="ob")
                    nc.scalar.activation(
                        ob[:], po[oh][:], Act.Identity, bias=bdc_sb[:, oh : oh + 1]
                    )
                    nc.sync.dma_start(
                        out[oh, :, s * SPX : (s + 1) * SPX], ob[:]
                    )


def _build():
    if "nc" in _BUILT:
        return _BUILT["nc"]
    nc = bacc.Bacc(
        "TRN2",
        target_bir_lowering=False,
        debug=False,
        enable_asserts=False,
        num_devices=NCORES,
    )
    xt = nc.dram_tensor("xt", [NGTOK, 4 * C], BF, kind="ExternalInput").ap()
    xc = nc.dram_tensor("xc", [P, 2, PADH * PADW], BF, kind="ExternalInput").ap()
    wofl = nc.dram_tensor("wofl", [P, 2, K2, 18], BF, kind="ExternalInput").ap()
    boff = nc.dram_tensor("boff", [18, 1], F32, kind="ExternalInput").ap()
    wdcl = nc.dram_tensor("wdcl", [P, K2, 2, 2, P], BF, kind="ExternalInput").ap()
    bdc = nc.dram_tensor("bdc", [P, 2], F32, kind="ExternalInput").ap()
    pyb = nc.dram_tensor("pyb", [P, NTILE, K2], F32, kind="ExternalInput").ap()
    pxb = nc.dram_tensor("pxb", [P, NTILE, K2], F32, kind="ExternalInput").ap()
    out = nc.dram_tensor("out", [2, P, NPIX], F32, kind="ExternalOutput").ap()
    with tile.TileContext(nc) as tc:
        _emit(tc, nc, (xt, xc, wofl, boff, wdcl, bdc, pyb, pxb, out))
    nc.compile()
    _BUILT["nc"] = nc
    return nc


def _build_xu(xs):
    """2x2-corner patch table for one sample: U[t] = (x[t-96], x[t-95],
    x[t], x[t+1]) over the token-major [NTOK, C] view, zero-padded."""
    import ml_dtypes

    bf16 = ml_dtypes.bfloat16
    xt = xs.transpose(1, 2, 0).reshape(NTOK, C).astype(bf16)
    xe = np.zeros((NGTOK + W + 1, C), bf16)
    xe[W : W + NTOK] = xt
    xu = np.stack(
        [xe[0:NGTOK], xe[1 : 1 + NGTOK], xe[W : W + NGTOK], xe[W + 1 : W + 1 + NGTOK]],
        axis=1,
    )  # [NGTOK, 4, C]
    return xu.reshape(NGTOK, 4 * C)


def _prep_core(k, x, w_off, b_off, w_dc, b_dc, xu_cache):
    b, half = k // 2, k % 2
    y0 = half * ROWS
    xs = x[b]  # [C,H,W] f32
    if b not in xu_cache:
        xu_cache[b] = _build_xu(xs)
    xt = xu_cache[b]
    xc = np.zeros((C, PADH, PADW), np.float32)
    r0, r1 = max(0, y0 - 1), min(H, y0 + ROWS + 1)
    xc[:, (r0 - (y0 - 1)) : (r1 - (y0 - 1)), 1 : 1 + W] = xs[:, r0:r1, :]
    xc = xc.reshape(2, P, PADH * PADW).transpose(1, 0, 2)

    wofl = (
        w_off.reshape(2 * K2, 2, P, K2)   # [oc, chalf, c, tap]
        .transpose(2, 1, 3, 0)            # [c, chalf, tap, oc]
        .copy()
    )
    wdcl = (
        w_dc.reshape(2, P, 2, P, K2)      # [oh, o, chalf, c, tap]
        .transpose(3, 4, 2, 0, 1)         # [c, tap, chalf, oh, o]
        .copy()
    )
    bdc = b_dc.reshape(2, P).transpose(1, 0).copy()

    pp = np.arange(NPIX)
    yg = y0 + pp // W
    xg = pp % W
    ti = (np.arange(K2) // K)[None, :]
    tj = (np.arange(K2) % K)[None, :]
    pyb = (yg[:, None] - 1 + ti).astype(np.float32).reshape(NTILE, P, K2)
    pxb = (xg[:, None] - 1 + tj).astype(np.float32).reshape(NTILE, P, K2)

    import ml_dtypes

    bf16 = ml_dtypes.bfloat16
    return {
        "xt": xt,
        "xc": xc.astype(bf16),
        "wofl": wofl.astype(bf16),
        "boff": b_off.reshape(18, 1).astype(np.float32),
        "wdcl": wdcl.astype(bf16),
        "bdc": bdc.astype(np.float32),
        "pyb": pyb.transpose(1, 0, 2).copy(),
        "pxb": pxb.transpose(1, 0, 2).copy(),
    }


def kernel(x, w_off, b_off, w_dc, b_dc, _trace=False):
    nc = _build()
    x = np.asarray(x, np.float32)
    w_off = np.asarray(w_off, np.float32)
    b_off = np.asarray(b_off, np.float32)
    w_dc = np.asarray(w_dc, np.float32)
    b_dc = np.asarray(b_dc, np.float32)
    xu_cache = {}
    in_maps = [
        _prep_core(k, x, w_off, b_off, w_dc, b_dc, xu_cache)
        for k in range(NCORES)
    ]
    res = bass_utils.run_bass_kernel_spmd(
        nc, in_maps, core_ids=list(range(NCORES)), trace=_trace
    )
    out = np.empty((B, O, H, W), np.float32)
    for k in range(NCORES):
        b, half = k // 2, k % 2
        o = res.results[k]["out"]  # [2,128,4608]
        out[b, :, half * ROWS : (half + 1) * ROWS, :] = o.reshape(
            O, ROWS, W
        )
    if _trace:
        return out, res
    return out



# revision 23
# speedup vs baseline: 2.5477x; 1.5544x over previous
"""Deformable conv (3x3, with offset-predicting conv) for Trainium2, 8 cores.

Sharding: pure data parallel. Core k handles sample b = k//2, output row block
(k%2)*48 .. +48 (48 rows x 96 cols = 4608 pixels). Full sample's x is available
to every core as a DRAM token table, so gathers are purely local.

Per-core pipeline (all on one NeuronCore, scheduled by Tile):
  A. offset conv (3x3, C=256 -> 18) as 18 PE matmuls per 4-row chunk
  B. PE-transpose offsets into pixel-partition layout [128px, tile, 18]
  C. DVE index/weight math: bilinear corner weights (zero-pad semantics exactly
     like the reference) + int16 gather token indices
  D. fold indices into the SWDGE "wrapped 16-partition" layout + replicate x8
  E. per (stage of 512 px, tap): dma_gather from a host-built 2x2-patch table
     U[t] = (x[t-96], x[t-95], x[t], x[t+1]) -- ONE 2KB descriptor per
     (pixel, tap) fetching all 4 bilinear corners -> [128px, tile, 4*256];
     DVE 4-term FMA combine with per-partition (per-pixel) scalar weights;
     PE transpose to channel layout; PE matmul accumulating over (c,tap)
     into PSUM [o,px]; bias + store.
"""

import dataclasses

import numpy as np

import concourse.bacc as bacc
import concourse.bass as bass
import concourse.mybir as mybir
import concourse.tile as tile
from concourse import bass_utils, masks
from concourse.mybir import ActivationFunctionType as Act
from concourse.mybir import AluOpType as Op

P = 128
B, C, H, W, O = 4, 256, 96, 96, 256
K = 3
K2 = 9
NCORES = 8
ROWS = 48                      # output rows per core
NPIX = ROWS * W                # 4608
NTILE = NPIX // P              # 36 pixel tiles of 128
NSTAGE = 9                     # stages of 512 px
TPS = 4                        # pixel tiles per stage
SPX = TPS * P                  # 512
PADH, PADW = ROWS + 2, W + 2   # 50, 98
NTOK = H * W                   # 9216
NGTOK = NTOK + W               # 9312 patch-table entries (row base -1..96)
CONV_ROWS_PER_CHUNK = 4        # offset-conv N chunk = 4 rows = 384 cols
NCHUNK = ROWS // CONV_ROWS_PER_CHUNK  # 12
BF = mybir.dt.bfloat16
F32 = mybir.dt.float32
I16 = mybir.dt.int16

_BUILT = {}


def _emit(tc, nc, io):
    xt, xc, wofl, boff, wdcl, bdc, pyb, pxb, out = io

    with (
        tc.tile_pool(name="const", bufs=1) as cpool,
        tc.tile_pool(name="sbig", bufs=1) as spool,
    ):
        ident_bf = cpool.tile([P, P], BF, tag="idbf", name="idbf")
        ident_f = cpool.tile([P, P], F32, tag="idf", name="idf")
        masks.make_identity(nc, ident_bf[:])
        masks.make_identity(nc, ident_f[:])

        # ---- persistent SBUF buffers ----
        xc_sb = spool.tile([P, 2, PADH * PADW], BF, tag="xc", name="xc")      # 19.6KB/part
        wofl_sb = spool.tile([P, 2, K2, 18], BF, tag="wofl", name="wofl")
        wdcl_sb = spool.tile([P, K2, 2, 2, P], BF, tag="wdcl", name="wdcl")     # 9.2KB/part
        boff_sb = spool.tile([18, 1], F32, tag="boff", name="boff")
        bdc_sb = spool.tile([P, 2], F32, tag="bdc", name="bdc")
        pyb_sb = spool.tile([P, K2, NTILE], F32, tag="pyb", name="pyb")
        pxb_sb = spool.tile([P, K2, NTILE], F32, tag="pxb", name="pxb")
        off_sb = spool.tile([18, NPIX], F32, tag="off", name="off")            # 18 parts
        doff = spool.tile([P, NTILE, 18], F32, tag="doff", name="doff")
        wt = spool.tile([P, K2, NTILE, 4], F32, tag="wt", name="wt")          # corner wgts
        idxn = spool.tile([P, K2, NTILE], F32, tag="idxn", name="idxn")         # f=(tap,t)
        twrap = spool.tile([P, K2 * NTILE * 8], I16, tag="twrap", name="twrap")  # 5.2KB

        nc.sync.dma_start(xc_sb[:], xc)
        nc.sync.dma_start(wofl_sb[:], wofl)
        nc.sync.dma_start(wdcl_sb[:], wdcl)
        nc.sync.dma_start(boff_sb[:], boff)
        nc.sync.dma_start(bdc_sb[:], bdc)
        nc.sync.dma_start(pyb_sb[:], pyb)
        nc.sync.dma_start(pxb_sb[:], pxb)

        # ---- A: offset conv ----
        with tc.tile_pool(name="psA", bufs=2, space="PSUM") as psa:
            for ch_i in range(NCHUNK):
                ncols = CONV_ROWS_PER_CHUNK * W  # 384
                ps = psa.tile([18, ncols], F32, tag="psoff", name="psoff")
                n_mm = 2 * K2
                mm = 0
                xcf = xc_sb[:]
                for chalf in range(2):
                    for tap in range(K2):
                        ti, tj = tap // K, tap % K
                        rhs = dataclasses.replace(
                            xcf,
                            ap=[
                                [xcf.ap[0][0], P],
                                [PADW, CONV_ROWS_PER_CHUNK],
                                [1, W],
                            ],
                            offset=xcf.offset
                            + chalf * (PADH * PADW)
                            + ((ch_i * CONV_ROWS_PER_CHUNK + ti) * PADW + tj),
                        )
                        nc.tensor.matmul(
                            ps[:],
                            wofl_sb[:, chalf, tap],
                            rhs,
                            start=(mm == 0),
                            stop=(mm == n_mm - 1),
                        )
                        mm += 1
                nc.scalar.activation(
                    off_sb[:, ch_i * ncols : (ch_i + 1) * ncols],
                    ps[:],
                    Act.Identity,
                    bias=boff_sb[:],
                )

        # ---- B: transpose offsets to pixel layout ----
        with tc.tile_pool(name="psB", bufs=4, space="PSUM") as psb:
            for t in range(NTILE):
                pt = psb.tile([P, 18], F32, tag="pofft", name="pofft")
                nc.tensor.transpose(
                    pt[:], off_sb[:, t * P : (t + 1) * P], ident_f[:18, :18]
                )
                nc.scalar.copy(doff[:, t, :], pt[:])

        # ---- C: index / weight math (DVE over [128, 9*36], f=(tap,t)) ----
        with tc.tile_pool(name="scr", bufs=1) as scr:
            sh = [P, K2, NTILE]

            def tmp(tag):
                return scr.tile(sh, F32, tag=tag, name=tag)

            # py16/px16 = sample coords + 16 (strictly positive); y0/x0 here
            # are floor(py)+16 etc. All downstream constants are shifted +16.
            MAGIC = 8388608.0  # 2^23
            dy = doff[:, :, 0:18:2].transpose((0, 2, 1))
            dx = doff[:, :, 1:18:2].transpose((0, 2, 1))
            py = tmp("py")
            px = tmp("px")
            nc.vector.tensor_tensor(py[:], pyb_sb[:], dy, Op.add)
            nc.vector.tensor_tensor(px[:], pxb_sb[:], dx, Op.add)
            nc.vector.tensor_scalar(py[:], py[:], 16.0, None, Op.add)
            nc.vector.tensor_scalar(px[:], px[:], 16.0, None, Op.add)
            y0 = tmp("y0")
            x0 = tmp("x0")
            nc.vector.tensor_scalar(y0[:], py[:], -0.4999999, None, Op.add)
            nc.vector.tensor_scalar(y0[:], y0[:], MAGIC, -MAGIC, Op.add, Op.add)
            nc.vector.tensor_scalar(x0[:], px[:], -0.4999999, None, Op.add)
            nc.vector.tensor_scalar(x0[:], x0[:], MAGIC, -MAGIC, Op.add, Op.add)
            ly = tmp("ly")
            lx = tmp("lx")
            nc.vector.tensor_tensor(ly[:], py[:], y0[:], Op.subtract)
            nc.vector.tensor_tensor(lx[:], px[:], x0[:], Op.subtract)

            ta_ = tmp("ta")
            tb_ = tmp("tb")
            tc_ = tmp("tc")
            td_ = tmp("td")
            # y weights: wy0 = (1-ly)*[0<=y0<=95], wy1 = ly*[0<=y0+1<=95]
            # (all bounds shifted +16)
            nc.vector.tensor_scalar(ta_[:], y0[:], 16.0, None, Op.is_ge)
            nc.vector.tensor_scalar(tb_[:], y0[:], 111.0, None, Op.is_le)
            vy0 = tmp("vy0")
            nc.vector.tensor_tensor(vy0[:], ta_[:], tb_[:], Op.mult)
            nc.vector.tensor_scalar(ta_[:], y0[:], 15.0, None, Op.is_ge)
            nc.vector.tensor_scalar(tb_[:], y0[:], 110.0, None, Op.is_le)
            vy1 = tmp("vy1")
            nc.vector.tensor_tensor(vy1[:], ta_[:], tb_[:], Op.mult)
            wy0 = tmp("wy0")
            wy1 = tmp("wy1")
            nc.vector.tensor_scalar(tc_[:], ly[:], -1.0, 1.0, Op.mult, Op.add)
            nc.vector.tensor_tensor(wy0[:], tc_[:], vy0[:], Op.mult)
            nc.vector.tensor_tensor(wy1[:], ly[:], vy1[:], Op.mult)

            # x pair weights on tokens (xb, xb+1), xb = clip(x0,0,95):
            # wA = (1-lx)*[0<=x0<=95] + lx*[x0==-1] ; wB = lx*[0<=x0<=94]
            # (all bounds shifted +16)
            nc.vector.tensor_scalar(ta_[:], x0[:], 16.0, None, Op.is_ge)
            nc.vector.tensor_scalar(tb_[:], x0[:], 111.0, None, Op.is_le)
            vx = tmp("vx")
            nc.vector.tensor_tensor(vx[:], ta_[:], tb_[:], Op.mult)
            nc.vector.tensor_scalar(tb_[:], x0[:], 110.0, None, Op.is_le)
            vxb = tmp("vxb")
            nc.vector.tensor_tensor(vxb[:], ta_[:], tb_[:], Op.mult)
            nc.vector.tensor_scalar(td_[:], x0[:], 15.0, None, Op.is_equal)
            wa = tmp("wa")
            wb = tmp("wb")
            nc.vector.tensor_scalar(tc_[:], lx[:], -1.0, 1.0, Op.mult, Op.add)
            nc.vector.tensor_tensor(tc_[:], tc_[:], vx[:], Op.mult)
            nc.vector.tensor_tensor(td_[:], lx[:], td_[:], Op.mult)
            nc.vector.tensor_tensor(wa[:], tc_[:], td_[:], Op.add)
            nc.vector.tensor_tensor(wb[:], lx[:], vxb[:], Op.mult)

            # final 4 corner weights -> wt[:, tap, t, 0..3] = A0,B0,A1,B1
            nc.vector.tensor_tensor(wt[:, :, :, 0], wy0[:], wa[:], Op.mult)
            nc.vector.tensor_tensor(wt[:, :, :, 1], wy0[:], wb[:], Op.mult)
            nc.vector.tensor_tensor(wt[:, :, :, 2], wy1[:], wa[:], Op.mult)
            nc.vector.tensor_tensor(wt[:, :, :, 3], wy1[:], wb[:], Op.mult)

            # patch-table index: idx = clip(y0+1,0,96)*96 + clip(x0,0,95);
            # table entry t holds corners (rows t//96-1, t//96; cols t%96,
            # t%96+1). (y0/x0 carry +16: un-shift via the final constant)
            xb = tmp("xb")
            nc.vector.tensor_scalar(xb[:], x0[:], 16.0, 111.0, Op.max, Op.min)
            ybn = tmp("ybn")
            nc.vector.tensor_scalar(ybn[:], y0[:], 15.0, 111.0, Op.max, Op.min)
            nc.vector.scalar_tensor_tensor(
                idxn[:], ybn[:], 96.0, xb[:], Op.mult, Op.add
            )
            nc.vector.tensor_scalar(idxn[:], idxn[:], -1456.0, None, Op.add)

        # ---- D: fold idxn into wrapped layout twrap[16g+r, f*8+k] ----
        # twrap[r, f*8+k] = idxn[16k+r, f] (f = tap*36+t); done as a
        # transpose -> 64B-run interleave DMA -> transpose sandwich to avoid
        # 2B-granule descriptors. PE transpose is a bit-exact data mover.
        FD = K2 * NTILE  # 324
        FB = 12          # f-block per final transpose (96 = 12f x 8k parts)
        NBLK = FD // FB  # 27
        CH = 108         # f per first-stage transpose chunk (= 9 blocks)
        if True:
            with (
                tc.tile_pool(name="fold", bufs=1) as fpool,
                tc.tile_pool(name="psF", bufs=4, space="PSUM") as psf,
            ):
                it_sb = fpool.tile([CH, 3, P], F32, tag="it", name="it")
                m_sb = fpool.tile([P, NBLK, 16], F32, tag="m", name="m")
                ixf = idxn[:]
                pitch_ix = ixf.ap[0][0]
                for c in range(3):
                    pf = psf.tile([CH, P], F32, tag="pf", name="pf")
                    src_ix = dataclasses.replace(
                        ixf, ap=[[pitch_ix, P], [1, CH]], offset=ixf.offset + c * CH
                    )
                    nc.tensor.transpose(pf[:], src_ix, ident_f[:])
                    nc.scalar.copy(it_sb[:, c, :], pf[:])
                itf = it_sb[:]
                pitch_it = itf.ap[0][0]
                mf = m_sb[:]
                pitch_m = mf.ap[0][0]
                for b in range(NBLK):
                    c, fl0 = divmod(b * FB, CH)
                    src = dataclasses.replace(
                        itf,
                        ap=[[pitch_it, FB], [16, 8], [1, 16]],
                        offset=itf.offset + fl0 * pitch_it + c * P,
                    )
                    dst = dataclasses.replace(
                        mf,
                        ap=[[pitch_m, FB * 8], [1, 16]],
                        offset=mf.offset + b * 16,
                    )
                    nc.sync.dma_start(dst, src)
                for b in range(NBLK):
                    pf2 = psf.tile([16, 96], F32, tag="pf2", name="pf2")
                    nc.tensor.transpose(
                        pf2[:], m_sb[0:96, b, :], ident_f[:96, :96]
                    )
                    nc.vector.tensor_copy(
                        twrap[0:16, b * 96 : (b + 1) * 96], pf2[:]
                    )
        for g in range(1, 8):
            nc.sync.dma_start(twrap[16 * g : 16 * (g + 1), :], twrap[0:16, :])

        # ---- E: main loop ----
        with (
            tc.tile_pool(name="gpool", bufs=4) as gpool,
            tc.tile_pool(name="vpool", bufs=4) as vpool,
            tc.tile_pool(name="rpool", bufs=3) as rpool,
            tc.tile_pool(name="opool", bufs=3) as opool,
            tc.tile_pool(name="psT", bufs=4, space="PSUM") as pst,
            tc.tile_pool(name="psO", bufs=2, space="PSUM") as pso,
        ):
            # 2x2-patch table: entry t = 4 tokens x 256c (2KB)
            xt_ap = xt
            xt_win = dataclasses.replace(
                xt_ap, ap=[[4 * C, NGTOK], [1, 4 * C]], offset=0
            )
            for s in range(NSTAGE):
                po = [pso.tile([P, SPX], F32, tag=f"po{oh}", name=f"po{oh}") for oh in range(2)]
                for tap in range(K2):
                    g = gpool.tile([P, TPS, 4 * C], BF, tag="g", name="g")
                    idxs = twrap[:, (tap * NTILE + s * TPS) * 8
                                 : (tap * NTILE + (s + 1) * TPS) * 8]
                    nc.gpsimd.dma_gather(
                        g[:],
                        xt_win,
                        idxs,
                        SPX,
                        SPX,
                        elem_size=4 * C,
                        elem_step=4 * C,
                    )
                    rst = [rpool.tile([P, SPX], BF, tag=f"r{c}", name=f"r{c}") for c in range(2)]
                    for t in range(TPS):
                        v = vpool.tile([P, C], BF, tag="v", name="v")
                        wcol = wt[:, tap, s * TPS + t, :]
                        nc.vector.tensor_scalar(
                            v[:], g[:, t, 0:C], wcol[:, 0:1], None, Op.mult
                        )
                        nc.vector.scalar_tensor_tensor(
                            v[:], g[:, t, C : 2 * C], wcol[:, 1:2], v[:],
                            Op.mult, Op.add,
                        )
                        nc.vector.scalar_tensor_tensor(
                            v[:], g[:, t, 2 * C : 3 * C], wcol[:, 2:3], v[:],
                            Op.mult, Op.add,
                        )
                        nc.vector.scalar_tensor_tensor(
                            v[:], g[:, t, 3 * C : 4 * C], wcol[:, 3:4], v[:],
                            Op.mult, Op.add,
                        )
                        for chalf in range(2):
                            ptr = pst.tile([P, P], BF, tag="ptr", name="ptr")
                            nc.tensor.transpose(
                                ptr[:],
                                v[:, chalf * P : (chalf + 1) * P],
                                ident_bf[:],
                            )
                            nc.scalar.copy(
                                rst[chalf][:, t * P : (t + 1) * P], ptr[:]
                            )
                    for chalf in range(2):
                        for oh in range(2):
                            nc.tensor.matmul(
                                po[oh][:],
                                wdcl_sb[:, tap, chalf, oh],
                                rst[chalf][:],
                                start=(tap == 0 and chalf == 0),
                                stop=(tap == K2 - 1 and chalf == 1),
                            )
                for oh in range(2):
                    ob = opool.tile([P, SPX], F32, tag="ob", name="ob")
                    nc.scalar.activation(
                        ob[:], po[oh][:], Act.Identity, bias=bdc_sb[:, oh : oh + 1]
                    )
                    nc.sync.dma_start(
                        out[oh, :, s * SPX : (s + 1) * SPX], ob[:]
                    )


def _build():
    if "nc" in _BUILT:
        return _BUILT["nc"]
    nc = bacc.Bacc(
        "TRN2",
        target_bir_lowering=False,
        debug=False,
        enable_asserts=False,
        num_devices=NCORES,
    )
    xt = nc.dram_tensor("xt", [NGTOK, 4 * C], BF, kind="ExternalInput").ap()
    xc = nc.dram_tensor("xc", [P, 2, PADH * PADW], BF, kind="ExternalInput").ap()
    wofl = nc.dram_tensor("wofl", [P, 2, K2, 18], BF, kind="ExternalInput").ap()
    boff = nc.dram_tensor("boff", [18, 1], F32, kind="ExternalInput").ap()
    wdcl = nc.dram_tensor("wdcl", [P, K2, 2, 2, P], BF, kind="ExternalInput").ap()
    bdc = nc.dram_tensor("bdc", [P, 2], F32, kind="ExternalInput").ap()
    pyb = nc.dram_tensor("pyb", [P, K2, NTILE], F32, kind="ExternalInput").ap()
    pxb = nc.dram_tensor("pxb", [P, K2, NTILE], F32, kind="ExternalInput").ap()
    out = nc.dram_tensor("out", [2, P, NPIX], F32, kind="ExternalOutput").ap()
    with tile.TileContext(nc) as tc:
        _emit(tc, nc, (xt, xc, wofl, boff, wdcl, bdc, pyb, pxb, out))
    nc.compile()
    _BUILT["nc"] = nc
    return nc


def _build_xu(xs):
    """2x2-corner patch table for one sample: U[t] = (x[t-96], x[t-95],
    x[t], x[t+1]) over the token-major [NTOK, C] view, zero-padded."""
    import ml_dtypes

    bf16 = ml_dtypes.bfloat16
    xt = xs.transpose(1, 2, 0).reshape(NTOK, C).astype(bf16)
    xe = np.zeros((NGTOK + W + 1, C), bf16)
    xe[W : W + NTOK] = xt
    xu = np.stack(
        [xe[0:NGTOK], xe[1 : 1 + NGTOK], xe[W : W + NGTOK], xe[W + 1 : W + 1 + NGTOK]],
        axis=1,
    )  # [NGTOK, 4, C]
    return xu.reshape(NGTOK, 4 * C)


def _prep_core(k, x, w_off, b_off, w_dc, b_dc, xu_cache):
    b, half = k // 2, k % 2
    y0 = half * ROWS
    xs = x[b]  # [C,H,W] f32
    if b not in xu_cache:
        xu_cache[b] = _build_xu(xs)
    xt = xu_cache[b]
    xc = np.zeros((C, PADH, PADW), np.float32)
    r0, r1 = max(0, y0 - 1), min(H, y0 + ROWS + 1)
    xc[:, (r0 - (y0 - 1)) : (r1 - (y0 - 1)), 1 : 1 + W] = xs[:, r0:r1, :]
    xc = xc.reshape(2, P, PADH * PADW).transpose(1, 0, 2)

    wofl = (
        w_off.reshape(2 * K2, 2, P, K2)   # [oc, chalf, c, tap]
        .transpose(2, 1, 3, 0)            # [c, chalf, tap, oc]
        .copy()
    )
    wdcl = (
        w_dc.reshape(2, P, 2, P, K2)      # [oh, o, chalf, c, tap]
        .transpose(3, 4, 2, 0, 1)         # [c, tap, chalf, oh, o]
        .copy()
    )
    bdc = b_dc.reshape(2, P).transpose(1, 0).copy()

    pp = np.arange(NPIX)
    yg = y0 + pp // W
    xg = pp % W
    ti = (np.arange(K2) // K)[None, :]
    tj = (np.arange(K2) % K)[None, :]
    pyb = (yg[:, None] - 1 + ti).astype(np.float32).reshape(NTILE, P, K2)
    pxb = (xg[:, None] - 1 + tj).astype(np.float32).reshape(NTILE, P, K2)

    import ml_dtypes

    bf16 = ml_dtypes.bfloat16
    return {
        "xt": xt,
        "xc": xc.astype(bf16),
        "wofl": wofl.astype(bf16),
        "boff": b_off.reshape(18, 1).astype(np.float32),
        "wdcl": wdcl.astype(bf16),
        "bdc": bdc.astype(np.float32),
        "pyb": pyb.transpose(1, 2, 0).copy(),
        "pxb": pxb.transpose(1, 2, 0).copy(),
    }


def kernel(x, w_off, b_off, w_dc, b_dc, _trace=False):
    nc = _build()
    x = np.asarray(x, np.float32)
    w_off = np.asarray(w_off, np.float32)
    b_off = np.asarray(b_off, np.float32)
    w_dc = np.asarray(w_dc, np.float32)
    b_dc = np.asarray(b_dc, np.float32)
    xu_cache = {}
    in_maps = [
        _prep_core(k, x, w_off, b_off, w_dc, b_dc, xu_cache)
        for k in range(NCORES)
    ]
    res = bass_utils.run_bass_kernel_spmd(
        nc, in_maps, core_ids=list(range(NCORES)), trace=_trace
    )
    out = np.empty((B, O, H, W), np.float32)
    for k in range(NCORES):
        b, half = k // 2, k % 2
        o = res.results[k]["out"]  # [2,128,4608]
        out[b, :, half * ROWS : (half + 1) * ROWS, :] = o.reshape(
            O, ROWS, W
        )
    if _trace:
        return out, res
    return out



# revision 24
# speedup vs baseline: 2.8744x; 1.1283x over previous
"""Deformable conv (3x3, with offset-predicting conv) for Trainium2, 8 cores.

Sharding: pure data parallel. Core k handles sample b = k//2, output row block
(k%2)*48 .. +48 (48 rows x 96 cols = 4608 pixels). Full sample's x is available
to every core as a DRAM token table, so gathers are purely local.

Per-core pipeline (all on one NeuronCore, scheduled by Tile):
  A. offset conv (3x3, C=256 -> 18) as 18 PE matmuls per 4-row chunk
  B. PE-transpose offsets into pixel-partition layout [128px, tile, 18]
  C. DVE index/weight math: bilinear corner weights (zero-pad semantics exactly
     like the reference) + int16 gather token indices
  D. fold indices into the SWDGE "wrapped 16-partition" layout + replicate x8
  E. per (stage of 512 px, tap): dma_gather from a host-built 2x2-patch table
     U[t] = (x[t-96], x[t-95], x[t], x[t+1]) -- ONE 2KB descriptor per
     (pixel, tap) fetching all 4 bilinear corners -> [128px, tile, 4*256];
     DVE 4-term FMA combine with per-partition (per-pixel) scalar weights;
     PE transpose to channel layout; PE matmul accumulating over (c,tap)
     into PSUM [o,px]; bias + store.
"""

import dataclasses

import numpy as np

import concourse.bacc as bacc
import concourse.bass as bass
import concourse.mybir as mybir
import concourse.tile as tile
from concourse import bass_utils, masks
from concourse.mybir import ActivationFunctionType as Act
from concourse.mybir import AluOpType as Op

P = 128
B, C, H, W, O = 4, 256, 96, 96, 256
K = 3
K2 = 9
NCORES = 8
ROWS = 48                      # output rows per core
NPIX = ROWS * W                # 4608
NTILE = NPIX // P              # 36 pixel tiles of 128
NSTAGE = 9                     # stages of 512 px
TPS = 4                        # pixel tiles per stage
SPX = TPS * P                  # 512
PADH, PADW = ROWS + 2, W + 2   # 50, 98
NTOK = H * W                   # 9216
NGTOK = NTOK + W               # 9312 patch-table entries (row base -1..96)
CONV_ROWS_PER_CHUNK = 4        # offset-conv N chunk = 4 rows = 384 cols
NCHUNK = ROWS // CONV_ROWS_PER_CHUNK  # 12
BF = mybir.dt.bfloat16
F32 = mybir.dt.float32
I16 = mybir.dt.int16

_BUILT = {}


def _emit(tc, nc, io):
    xt, xc, wofl, boff, wdcl, bdc, pyb, pxb, out = io

    with (
        tc.tile_pool(name="const", bufs=1) as cpool,
        tc.tile_pool(name="sbig", bufs=1) as spool,
    ):
        ident_bf = cpool.tile([P, P], BF, tag="idbf", name="idbf")
        ident_f = cpool.tile([P, P], F32, tag="idf", name="idf")
        masks.make_identity(nc, ident_bf[:])
        masks.make_identity(nc, ident_f[:])

        # ---- persistent SBUF buffers ----
        xc_sb = spool.tile([P, 2, PADH * PADW], BF, tag="xc", name="xc")      # 19.6KB/part
        wofl_sb = spool.tile([P, 2, K2, 18], BF, tag="wofl", name="wofl")
        wdcl_sb = spool.tile([P, K2, 2, 2, P], BF, tag="wdcl", name="wdcl")     # 9.2KB/part
        boff_sb = spool.tile([18, 1], F32, tag="boff", name="boff")
        bdc_sb = spool.tile([P, 2], F32, tag="bdc", name="bdc")
        pyb_sb = spool.tile([P, K2, NTILE], F32, tag="pyb", name="pyb")
        pxb_sb = spool.tile([P, K2, NTILE], F32, tag="pxb", name="pxb")
        off_sb = spool.tile([18, NPIX], F32, tag="off", name="off")            # 18 parts
        doff = spool.tile([P, NTILE, 18], F32, tag="doff", name="doff")
        wt = spool.tile([P, K2, NTILE, 4], F32, tag="wt", name="wt")          # corner wgts
        idxn = spool.tile([P, K2, NTILE], F32, tag="idxn", name="idxn")         # f=(tap,t)
        twrap = spool.tile([P, K2 * NTILE * 8], I16, tag="twrap", name="twrap")  # 5.2KB

        nc.sync.dma_start(xc_sb[:], xc)
        nc.sync.dma_start(wofl_sb[:], wofl)
        nc.sync.dma_start(wdcl_sb[:], wdcl)
        nc.sync.dma_start(boff_sb[:], boff)
        nc.sync.dma_start(bdc_sb[:], bdc)
        nc.sync.dma_start(pyb_sb[:], pyb)
        nc.sync.dma_start(pxb_sb[:], pxb)

        # ---- A: offset conv ----
        with tc.tile_pool(name="psA", bufs=2, space="PSUM") as psa:
            for ch_i in range(NCHUNK):
                ncols = CONV_ROWS_PER_CHUNK * W  # 384
                ps = psa.tile([18, ncols], F32, tag="psoff", name="psoff")
                n_mm = 2 * K2
                mm = 0
                xcf = xc_sb[:]
                for chalf in range(2):
                    for tap in range(K2):
                        ti, tj = tap // K, tap % K
                        rhs = dataclasses.replace(
                            xcf,
                            ap=[
                                [xcf.ap[0][0], P],
                                [PADW, CONV_ROWS_PER_CHUNK],
                                [1, W],
                            ],
                            offset=xcf.offset
                            + chalf * (PADH * PADW)
                            + ((ch_i * CONV_ROWS_PER_CHUNK + ti) * PADW + tj),
                        )
                        nc.tensor.matmul(
                            ps[:],
                            wofl_sb[:, chalf, tap],
                            rhs,
                            start=(mm == 0),
                            stop=(mm == n_mm - 1),
                        )
                        mm += 1
                nc.scalar.activation(
                    off_sb[:, ch_i * ncols : (ch_i + 1) * ncols],
                    ps[:],
                    Act.Identity,
                    bias=boff_sb[:],
                )

        # ---- B: transpose offsets to pixel layout ----
        with tc.tile_pool(name="psB", bufs=4, space="PSUM") as psb:
            for t in range(NTILE):
                pt = psb.tile([P, 18], F32, tag="pofft", name="pofft")
                nc.tensor.transpose(
                    pt[:], off_sb[:, t * P : (t + 1) * P], ident_f[:18, :18]
                )
                nc.scalar.copy(doff[:, t, :], pt[:])

        # ---- C: index / weight math (DVE over [128, 9*36], f=(tap,t)) ----
        with tc.tile_pool(name="scr", bufs=1) as scr:
            sh = [P, K2, NTILE]

            def tmp(tag):
                return scr.tile(sh, F32, tag=tag, name=tag)

            # py16/px16 = sample coords + 16 (strictly positive); y0/x0 here
            # are floor(py)+16 etc. All downstream constants are shifted +16.
            MAGIC = 8388608.0  # 2^23
            dy = doff[:, :, 0:18:2].transpose((0, 2, 1))
            dx = doff[:, :, 1:18:2].transpose((0, 2, 1))
            py = tmp("py")
            px = tmp("px")
            nc.vector.tensor_tensor(py[:], pyb_sb[:], dy, Op.add)
            nc.vector.tensor_tensor(px[:], pxb_sb[:], dx, Op.add)
            nc.vector.tensor_scalar(py[:], py[:], 16.0, None, Op.add)
            nc.vector.tensor_scalar(px[:], px[:], 16.0, None, Op.add)
            y0 = tmp("y0")
            x0 = tmp("x0")
            nc.vector.tensor_scalar(y0[:], py[:], -0.4999999, None, Op.add)
            nc.vector.tensor_scalar(y0[:], y0[:], MAGIC, -MAGIC, Op.add, Op.add)
            nc.vector.tensor_scalar(x0[:], px[:], -0.4999999, None, Op.add)
            nc.vector.tensor_scalar(x0[:], x0[:], MAGIC, -MAGIC, Op.add, Op.add)
            ly = tmp("ly")
            lx = tmp("lx")
            nc.vector.tensor_tensor(ly[:], py[:], y0[:], Op.subtract)
            nc.vector.tensor_tensor(lx[:], px[:], x0[:], Op.subtract)

            ta_ = tmp("ta")
            tb_ = tmp("tb")
            tc_ = tmp("tc")
            td_ = tmp("td")
            # y weights: wy0 = (1-ly)*[0<=y0<=95], wy1 = ly*[0<=y0+1<=95]
            # (all bounds shifted +16)
            nc.vector.tensor_scalar(ta_[:], y0[:], 16.0, None, Op.is_ge)
            nc.vector.tensor_scalar(tb_[:], y0[:], 111.0, None, Op.is_le)
            vy0 = tmp("vy0")
            nc.vector.tensor_tensor(vy0[:], ta_[:], tb_[:], Op.mult)
            nc.vector.tensor_scalar(ta_[:], y0[:], 15.0, None, Op.is_ge)
            nc.vector.tensor_scalar(tb_[:], y0[:], 110.0, None, Op.is_le)
            vy1 = tmp("vy1")
            nc.vector.tensor_tensor(vy1[:], ta_[:], tb_[:], Op.mult)
            wy0 = tmp("wy0")
            wy1 = tmp("wy1")
            nc.vector.tensor_scalar(tc_[:], ly[:], -1.0, 1.0, Op.mult, Op.add)
            nc.vector.tensor_tensor(wy0[:], tc_[:], vy0[:], Op.mult)
            nc.vector.tensor_tensor(wy1[:], ly[:], vy1[:], Op.mult)

            # x pair weights on tokens (xb, xb+1), xb = clip(x0,0,95):
            # wA = (1-lx)*[0<=x0<=95] + lx*[x0==-1] ; wB = lx*[0<=x0<=94]
            # (all bounds shifted +16)
            nc.vector.tensor_scalar(ta_[:], x0[:], 16.0, None, Op.is_ge)
            nc.vector.tensor_scalar(tb_[:], x0[:], 111.0, None, Op.is_le)
            vx = tmp("vx")
            nc.vector.tensor_tensor(vx[:], ta_[:], tb_[:], Op.mult)
            nc.vector.tensor_scalar(tb_[:], x0[:], 110.0, None, Op.is_le)
            vxb = tmp("vxb")
            nc.vector.tensor_tensor(vxb[:], ta_[:], tb_[:], Op.mult)
            nc.vector.tensor_scalar(td_[:], x0[:], 15.0, None, Op.is_equal)
            wa = tmp("wa")
            wb = tmp("wb")
            nc.vector.tensor_scalar(tc_[:], lx[:], -1.0, 1.0, Op.mult, Op.add)
            nc.vector.tensor_tensor(tc_[:], tc_[:], vx[:], Op.mult)
            nc.vector.tensor_tensor(td_[:], lx[:], td_[:], Op.mult)
            nc.vector.tensor_tensor(wa[:], tc_[:], td_[:], Op.add)
            nc.vector.tensor_tensor(wb[:], lx[:], vxb[:], Op.mult)

            # final 4 corner weights -> wt[:, tap, t, 0..3] = A0,B0,A1,B1
            nc.vector.tensor_tensor(wt[:, :, :, 0], wy0[:], wa[:], Op.mult)
            nc.vector.tensor_tensor(wt[:, :, :, 1], wy0[:], wb[:], Op.mult)
            nc.vector.tensor_tensor(wt[:, :, :, 2], wy1[:], wa[:], Op.mult)
            nc.vector.tensor_tensor(wt[:, :, :, 3], wy1[:], wb[:], Op.mult)

            # patch-table index: idx = clip(y0+1,0,96)*96 + clip(x0,0,95);
            # table entry t holds corners (rows t//96-1, t//96; cols t%96,
            # t%96+1). (y0/x0 carry +16: un-shift via the final constant)
            xb = tmp("xb")
            nc.vector.tensor_scalar(xb[:], x0[:], 16.0, 111.0, Op.max, Op.min)
            ybn = tmp("ybn")
            nc.vector.tensor_scalar(ybn[:], y0[:], 15.0, 111.0, Op.max, Op.min)
            nc.vector.scalar_tensor_tensor(
                idxn[:], ybn[:], 96.0, xb[:], Op.mult, Op.add
            )
            nc.vector.tensor_scalar(idxn[:], idxn[:], -1456.0, None, Op.add)

        # ---- D: fold idxn into wrapped layout twrap[16g+r, f*8+k] ----
        # twrap[r, f*8+k] = idxn[16k+r, f] (f = tap*36+t); done as a
        # transpose -> 64B-run interleave DMA -> transpose sandwich to avoid
        # 2B-granule descriptors. PE transpose is a bit-exact data mover.
        FD = K2 * NTILE  # 324
        FB = 12          # f-block per final transpose (96 = 12f x 8k parts)
        NBLK = FD // FB  # 27
        CH = 108         # f per first-stage transpose chunk (= 9 blocks)
        if True:
            with (
                tc.tile_pool(name="fold", bufs=1) as fpool,
                tc.tile_pool(name="psF", bufs=4, space="PSUM") as psf,
            ):
                it_sb = fpool.tile([CH, 3, P], F32, tag="it", name="it")
                m_sb = fpool.tile([P, NBLK, 16], F32, tag="m", name="m")
                ixf = idxn[:]
                pitch_ix = ixf.ap[0][0]
                for c in range(3):
                    pf = psf.tile([CH, P], F32, tag="pf", name="pf")
                    src_ix = dataclasses.replace(
                        ixf, ap=[[pitch_ix, P], [1, CH]], offset=ixf.offset + c * CH
                    )
                    nc.tensor.transpose(pf[:], src_ix, ident_f[:])
                    nc.scalar.copy(it_sb[:, c, :], pf[:])
                itf = it_sb[:]
                pitch_it = itf.ap[0][0]
                mf = m_sb[:]
                pitch_m = mf.ap[0][0]
                for b in range(NBLK):
                    c, fl0 = divmod(b * FB, CH)
                    src = dataclasses.replace(
                        itf,
                        ap=[[pitch_it, FB], [16, 8], [1, 16]],
                        offset=itf.offset + fl0 * pitch_it + c * P,
                    )
                    dst = dataclasses.replace(
                        mf,
                        ap=[[pitch_m, FB * 8], [1, 16]],
                        offset=mf.offset + b * 16,
                    )
                    nc.sync.dma_start(dst, src)
                for b in range(NBLK):
                    pf2 = psf.tile([16, 96], F32, tag="pf2", name="pf2")
                    nc.tensor.transpose(
                        pf2[:], m_sb[0:96, b, :], ident_f[:96, :96]
                    )
                    nc.vector.tensor_copy(
                        twrap[0:16, b * 96 : (b + 1) * 96], pf2[:]
                    )
        for g in range(1, 8):
            nc.sync.dma_start(twrap[16 * g : 16 * (g + 1), :], twrap[0:16, :])

        # ---- E: main loop ----
        with (
            tc.tile_pool(name="gpool", bufs=4) as gpool,
            tc.tile_pool(name="vpool", bufs=4) as vpool,
            tc.tile_pool(name="rpool", bufs=3) as rpool,
            tc.tile_pool(name="opool", bufs=3) as opool,
            tc.tile_pool(name="psT", bufs=4, space="PSUM") as pst,
            tc.tile_pool(name="psO", bufs=2, space="PSUM") as pso,
        ):
            # 2x2-patch table: entry t = 4 tokens x 256c (2KB)
            xt_ap = xt
            xt_win = dataclasses.replace(
                xt_ap, ap=[[4 * C, NGTOK], [1, 4 * C]], offset=0
            )
            for s in range(NSTAGE):
                po = [pso.tile([P, SPX], F32, tag=f"po{oh}", name=f"po{oh}") for oh in range(2)]
                for tap in range(K2):
                    g = gpool.tile([P, TPS, 4 * C], BF, tag="g", name="g")
                    idxs = twrap[:, (tap * NTILE + s * TPS) * 8
                                 : (tap * NTILE + (s + 1) * TPS) * 8]
                    nc.gpsimd.dma_gather(
                        g[:],
                        xt_win,
                        idxs,
                        SPX,
                        SPX,
                        elem_size=4 * C,
                        elem_step=4 * C,
                    )
                    rst = [rpool.tile([P, SPX], BF, tag=f"r{c}", name=f"r{c}") for c in range(2)]
                    for t in range(TPS):
                        v = vpool.tile([P, C], BF, tag="v", name="v")
                        t2 = vpool.tile([P, C], BF, tag="t2", name="t2")
                        wcol = wt[:, tap, s * TPS + t, :]
                        # corner 3 on the scalar engine; DVE chains the rest
                        nc.scalar.activation(
                            t2[:], g[:, t, 3 * C : 4 * C], Act.Identity,
                            scale=wcol[:, 3:4],
                        )
                        nc.vector.scalar_tensor_tensor(
                            v[:], g[:, t, 0:C], wcol[:, 0:1], t2[:],
                            Op.mult, Op.add,
                        )
                        nc.vector.scalar_tensor_tensor(
                            v[:], g[:, t, C : 2 * C], wcol[:, 1:2], v[:],
                            Op.mult, Op.add,
                        )
                        nc.vector.scalar_tensor_tensor(
                            v[:], g[:, t, 2 * C : 3 * C], wcol[:, 2:3], v[:],
                            Op.mult, Op.add,
                        )
                        for chalf in range(2):
                            ptr = pst.tile([P, P], BF, tag="ptr", name="ptr")
                            nc.tensor.transpose(
                                ptr[:],
                                v[:, chalf * P : (chalf + 1) * P],
                                ident_bf[:],
                            )
                            nc.scalar.copy(
                                rst[chalf][:, t * P : (t + 1) * P], ptr[:]
                            )
                    for chalf in range(2):
                        for oh in range(2):
                            nc.tensor.matmul(
                                po[oh][:],
                                wdcl_sb[:, tap, chalf, oh],
                                rst[chalf][:],
                                start=(tap == 0 and chalf == 0),
                                stop=(tap == K2 - 1 and chalf == 1),
                            )
                for oh in range(2):
                    ob = opool.tile([P, SPX], F32, tag="ob", name="ob")
                    nc.scalar.activation(
                        ob[:], po[oh][:], Act.Identity, bias=bdc_sb[:, oh : oh + 1]
                    )
                    nc.sync.dma_start(
                        out[oh, :, s * SPX : (s + 1) * SPX], ob[:]
                    )


def _build():
    if "nc" in _BUILT:
        return _BUILT["nc"]
    nc = bacc.Bacc(
        "TRN2",
        target_bir_lowering=False,
        debug=False,
        enable_asserts=False,
        num_devices=NCORES,
    )
    xt = nc.dram_tensor("xt", [NGTOK, 4 * C], BF, kind="ExternalInput").ap()
    xc = nc.dram_tensor("xc", [P, 2, PADH * PADW], BF, kind="ExternalInput").ap()
    wofl = nc.dram_tensor("wofl", [P, 2, K2, 18], BF, kind="ExternalInput").ap()
    boff = nc.dram_tensor("boff", [18, 1], F32, kind="ExternalInput").ap()
    wdcl = nc.dram_tensor("wdcl", [P, K2, 2, 2, P], BF, kind="ExternalInput").ap()
    bdc = nc.dram_tensor("bdc", [P, 2], F32, kind="ExternalInput").ap()
    pyb = nc.dram_tensor("pyb", [P, K2, NTILE], F32, kind="ExternalInput").ap()
    pxb = nc.dram_tensor("pxb", [P, K2, NTILE], F32, kind="ExternalInput").ap()
    out = nc.dram_tensor("out", [2, P, NPIX], F32, kind="ExternalOutput").ap()
    with tile.TileContext(nc) as tc:
        _emit(tc, nc, (xt, xc, wofl, boff, wdcl, bdc, pyb, pxb, out))
    nc.compile()
    _BUILT["nc"] = nc
    return nc


def _build_xu(xs):
    """2x2-corner patch table for one sample: U[t] = (x[t-96], x[t-95],
    x[t], x[t+1]) over the token-major [NTOK, C] view, zero-padded."""
    import ml_dtypes

    bf16 = ml_dtypes.bfloat16
    xt = xs.transpose(1, 2, 0).reshape(NTOK, C).astype(bf16)
    xe = np.zeros((NGTOK + W + 1, C), bf16)
    xe[W : W + NTOK] = xt
    xu = np.stack(
        [xe[0:NGTOK], xe[1 : 1 + NGTOK], xe[W : W + NGTOK], xe[W + 1 : W + 1 + NGTOK]],
        axis=1,
    )  # [NGTOK, 4, C]
    return xu.reshape(NGTOK, 4 * C)


def _prep_core(k, x, w_off, b_off, w_dc, b_dc, xu_cache):
    b, half = k // 2, k % 2
    y0 = half * ROWS
    xs = x[b]  # [C,H,W] f32
    if b not in xu_cache:
        xu_cache[b] = _build_xu(xs)
    xt = xu_cache[b]
    xc = np.zeros((C, PADH, PADW), np.float32)
    r0, r1 = max(0, y0 - 1), min(H, y0 + ROWS + 1)
    xc[:, (r0 - (y0 - 1)) : (r1 - (y0 - 1)), 1 : 1 + W] = xs[:, r0:r1, :]
    xc = xc.reshape(2, P, PADH * PADW).transpose(1, 0, 2)

    wofl = (
        w_off.reshape(2 * K2, 2, P, K2)   # [oc, chalf, c, tap]
        .transpose(2, 1, 3, 0)            # [c, chalf, tap, oc]
        .copy()
    )
    wdcl = (
        w_dc.reshape(2, P, 2, P, K2)      # [oh, o, chalf, c, tap]
        .transpose(3, 4, 2, 0, 1)         # [c, tap, chalf, oh, o]
        .copy()
    )
    bdc = b_dc.reshape(2, P).transpose(1, 0).copy()

    pp = np.arange(NPIX)
    yg = y0 + pp // W
    xg = pp % W
    ti = (np.arange(K2) // K)[None, :]
    tj = (np.arange(K2) % K)[None, :]
    pyb = (yg[:, None] - 1 + ti).astype(np.float32).reshape(NTILE, P, K2)
    pxb = (xg[:, None] - 1 + tj).astype(np.float32).reshape(NTILE, P, K2)

    import ml_dtypes

    bf16 = ml_dtypes.bfloat16
    return {
        "xt": xt,
        "xc": xc.astype(bf16),
        "wofl": wofl.astype(bf16),
        "boff": b_off.reshape(18, 1).astype(np.float32),
        "wdcl": wdcl.astype(bf16),
        "bdc": bdc.astype(np.float32),
        "pyb": pyb.transpose(1, 2, 0).copy(),
        "pxb": pxb.transpose(1, 2, 0).copy(),
    }


def kernel(x, w_off, b_off, w_dc, b_dc, _trace=False):
    nc = _build()
    x = np.asarray(x, np.float32)
    w_off = np.asarray(w_off, np.float32)
    b_off = np.asarray(b_off, np.float32)
    w_dc = np.asarray(w_dc, np.float32)
    b_dc = np.asarray(b_dc, np.float32)
    xu_cache = {}
    in_maps = [
        _prep_core(k, x, w_off, b_off, w_dc, b_dc, xu_cache)
        for k in range(NCORES)
    ]
    res = bass_utils.run_bass_kernel_spmd(
        nc, in_maps, core_ids=list(range(NCORES)), trace=_trace
    )
    out = np.empty((B, O, H, W), np.float32)
    for k in range(NCORES):
        b, half = k // 2, k % 2
        o = res.results[k]["out"]  # [2,128,4608]
        out[b, :, half * ROWS : (half + 1) * ROWS, :] = o.reshape(
            O, ROWS, W
        )
    if _trace:
        return out, res
    return out



# revision 25
# speedup vs baseline: 2.8999x; 1.0089x over previous
"""Deformable conv (3x3, with offset-predicting conv) for Trainium2, 8 cores.

Sharding: pure data parallel. Core k handles sample b = k//2, output row block
(k%2)*48 .. +48 (48 rows x 96 cols = 4608 pixels). Full sample's x is available
to every core as a DRAM token table, so gathers are purely local.

Per-core pipeline (all on one NeuronCore, scheduled by Tile):
  A. offset conv (3x3, C=256 -> 18) as 18 PE matmuls per 4-row chunk
  B. PE-transpose offsets into pixel-partition layout [128px, tile, 18]
  C. DVE index/weight math: bilinear corner weights (zero-pad semantics exactly
     like the reference) + int16 gather token indices
  D. fold indices into the SWDGE "wrapped 16-partition" layout + replicate x8
  E. per (stage of 512 px, tap): dma_gather from a host-built 2x2-patch table
     U[t] = (x[t-96], x[t-95], x[t], x[t+1]) -- ONE 2KB descriptor per
     (pixel, tap) fetching all 4 bilinear corners -> [128px, tile, 4*256];
     DVE 4-term FMA combine with per-partition (per-pixel) scalar weights;
     PE transpose to channel layout; PE matmul accumulating over (c,tap)
     into PSUM [o,px]; bias + store.
"""

import dataclasses

import numpy as np

import concourse.bacc as bacc
import concourse.bass as bass
import concourse.mybir as mybir
import concourse.tile as tile
from concourse import bass_utils, masks
from concourse.mybir import ActivationFunctionType as Act
from concourse.mybir import AluOpType as Op

P = 128
B, C, H, W, O = 4, 256, 96, 96, 256
K = 3
K2 = 9
NCORES = 8
ROWS = 48                      # output rows per core
NPIX = ROWS * W                # 4608
NTILE = NPIX // P              # 36 pixel tiles of 128
NSTAGE = 9                     # stages of 512 px
TPS = 4                        # pixel tiles per stage
SPX = TPS * P                  # 512
PADH, PADW = ROWS + 2, W + 2   # 50, 98
NTOK = H * W                   # 9216
NGTOK = NTOK + W               # 9312 patch-table entries (row base -1..96)
CONV_ROWS_PER_CHUNK = 4        # offset-conv N chunk = 4 rows = 384 cols
NCHUNK = ROWS // CONV_ROWS_PER_CHUNK  # 12
BF = mybir.dt.bfloat16
F32 = mybir.dt.float32
I16 = mybir.dt.int16

_BUILT = {}


def _emit(tc, nc, io):
    xt, xc, wofl, boff, wdcl, bdc, pyb, pxb, out = io

    with (
        tc.tile_pool(name="const", bufs=1) as cpool,
        tc.tile_pool(name="sbig", bufs=1) as spool,
    ):
        ident_bf = cpool.tile([P, P], BF, tag="idbf", name="idbf")
        ident_f = cpool.tile([P, P], F32, tag="idf", name="idf")
        masks.make_identity(nc, ident_bf[:])
        masks.make_identity(nc, ident_f[:])

        # ---- persistent SBUF buffers ----
        xc_sb = spool.tile([P, 2, PADH * PADW], BF, tag="xc", name="xc")      # 19.6KB/part
        wofl_sb = spool.tile([P, 2, K2, 18], BF, tag="wofl", name="wofl")
        wdcl_sb = spool.tile([P, K2, 2, 2, P], BF, tag="wdcl", name="wdcl")     # 9.2KB/part
        boff_sb = spool.tile([18, 1], F32, tag="boff", name="boff")
        bdc_sb = spool.tile([P, 2], F32, tag="bdc", name="bdc")
        pyb_sb = spool.tile([P, K2, NTILE], F32, tag="pyb", name="pyb")
        pxb_sb = spool.tile([P, K2, NTILE], F32, tag="pxb", name="pxb")
        off_sb = spool.tile([18, NPIX], F32, tag="off", name="off")            # 18 parts
        doff = spool.tile([P, NTILE, 18], F32, tag="doff", name="doff")
        wt = spool.tile([P, K2, NTILE, 4], F32, tag="wt", name="wt")          # corner wgts
        idxn = spool.tile([P, K2, NTILE], F32, tag="idxn", name="idxn")         # f=(tap,t)
        twrap = spool.tile([P, K2 * NTILE * 8], I16, tag="twrap", name="twrap")  # 5.2KB

        nc.sync.dma_start(xc_sb[:], xc)
        nc.sync.dma_start(wofl_sb[:], wofl)
        nc.sync.dma_start(wdcl_sb[:], wdcl)
        nc.sync.dma_start(boff_sb[:], boff)
        nc.sync.dma_start(bdc_sb[:], bdc)
        nc.sync.dma_start(pyb_sb[:], pyb)
        nc.sync.dma_start(pxb_sb[:], pxb)

        # ---- A: offset conv ----
        with tc.tile_pool(name="psA", bufs=2, space="PSUM") as psa:
            for ch_i in range(NCHUNK):
                ncols = CONV_ROWS_PER_CHUNK * W  # 384
                ps = psa.tile([18, ncols], F32, tag="psoff", name="psoff")
                n_mm = 2 * K2
                mm = 0
                xcf = xc_sb[:]
                for chalf in range(2):
                    for tap in range(K2):
                        ti, tj = tap // K, tap % K
                        rhs = dataclasses.replace(
                            xcf,
                            ap=[
                                [xcf.ap[0][0], P],
                                [PADW, CONV_ROWS_PER_CHUNK],
                                [1, W],
                            ],
                            offset=xcf.offset
                            + chalf * (PADH * PADW)
                            + ((ch_i * CONV_ROWS_PER_CHUNK + ti) * PADW + tj),
                        )
                        nc.tensor.matmul(
                            ps[:],
                            wofl_sb[:, chalf, tap],
                            rhs,
                            start=(mm == 0),
                            stop=(mm == n_mm - 1),
                        )
                        mm += 1
                nc.scalar.activation(
                    off_sb[:, ch_i * ncols : (ch_i + 1) * ncols],
                    ps[:],
                    Act.Identity,
                    bias=boff_sb[:],
                )

        # ---- B: transpose offsets to pixel layout ----
        with tc.tile_pool(name="psB", bufs=4, space="PSUM") as psb:
            for t in range(NTILE):
                pt = psb.tile([P, 18], F32, tag="pofft", name="pofft")
                nc.tensor.transpose(
                    pt[:], off_sb[:, t * P : (t + 1) * P], ident_f[:18, :18]
                )
                nc.scalar.copy(doff[:, t, :], pt[:])

        # ---- C: index / weight math (DVE over [128, 9*36], f=(tap,t)) ----
        with tc.tile_pool(name="scr", bufs=1) as scr:
            sh = [P, K2, NTILE]

            def tmp(tag):
                return scr.tile(sh, F32, tag=tag, name=tag)

            # py16/px16 = sample coords + 16 (strictly positive); y0/x0 here
            # are floor(py)+16 etc. All downstream constants are shifted +16.
            MAGIC = 8388608.0  # 2^23
            dy = doff[:, :, 0:18:2].transpose((0, 2, 1))
            dx = doff[:, :, 1:18:2].transpose((0, 2, 1))
            py = tmp("py")
            px = tmp("px")
            nc.vector.tensor_tensor(py[:], pyb_sb[:], dy, Op.add)
            nc.vector.tensor_tensor(px[:], pxb_sb[:], dx, Op.add)
            nc.vector.tensor_scalar(py[:], py[:], 16.0, None, Op.add)
            nc.vector.tensor_scalar(px[:], px[:], 16.0, None, Op.add)
            y0 = tmp("y0")
            x0 = tmp("x0")
            nc.vector.tensor_scalar(y0[:], py[:], -0.4999999, None, Op.add)
            nc.vector.tensor_scalar(y0[:], y0[:], MAGIC, -MAGIC, Op.add, Op.add)
            nc.vector.tensor_scalar(x0[:], px[:], -0.4999999, None, Op.add)
            nc.vector.tensor_scalar(x0[:], x0[:], MAGIC, -MAGIC, Op.add, Op.add)
            ly = tmp("ly")
            lx = tmp("lx")
            nc.vector.tensor_tensor(ly[:], py[:], y0[:], Op.subtract)
            nc.vector.tensor_tensor(lx[:], px[:], x0[:], Op.subtract)

            ta_ = tmp("ta")
            tb_ = tmp("tb")
            tc_ = tmp("tc")
            td_ = tmp("td")
            # y weights: wy0 = (1-ly)*[0<=y0<=95], wy1 = ly*[0<=y0+1<=95]
            # (all bounds shifted +16)
            nc.vector.tensor_scalar(ta_[:], y0[:], 16.0, None, Op.is_ge)
            nc.vector.tensor_scalar(tb_[:], y0[:], 111.0, None, Op.is_le)
            vy0 = tmp("vy0")
            nc.vector.tensor_tensor(vy0[:], ta_[:], tb_[:], Op.mult)
            nc.vector.tensor_scalar(ta_[:], y0[:], 15.0, None, Op.is_ge)
            nc.vector.tensor_scalar(tb_[:], y0[:], 110.0, None, Op.is_le)
            vy1 = tmp("vy1")
            nc.vector.tensor_tensor(vy1[:], ta_[:], tb_[:], Op.mult)
            wy0 = tmp("wy0")
            wy1 = tmp("wy1")
            nc.vector.tensor_scalar(tc_[:], ly[:], -1.0, 1.0, Op.mult, Op.add)
            nc.vector.tensor_tensor(wy0[:], tc_[:], vy0[:], Op.mult)
            nc.vector.tensor_tensor(wy1[:], ly[:], vy1[:], Op.mult)

            # x pair weights on tokens (xb, xb+1), xb = clip(x0,0,95):
            # wA = (1-lx)*[0<=x0<=95] + lx*[x0==-1] ; wB = lx*[0<=x0<=94]
            # (all bounds shifted +16)
            nc.vector.tensor_scalar(ta_[:], x0[:], 16.0, None, Op.is_ge)
            nc.vector.tensor_scalar(tb_[:], x0[:], 111.0, None, Op.is_le)
            vx = tmp("vx")
            nc.vector.tensor_tensor(vx[:], ta_[:], tb_[:], Op.mult)
            nc.vector.tensor_scalar(tb_[:], x0[:], 110.0, None, Op.is_le)
            vxb = tmp("vxb")
            nc.vector.tensor_tensor(vxb[:], ta_[:], tb_[:], Op.mult)
            nc.vector.tensor_scalar(td_[:], x0[:], 15.0, None, Op.is_equal)
            wa = tmp("wa")
            wb = tmp("wb")
            nc.vector.tensor_scalar(tc_[:], lx[:], -1.0, 1.0, Op.mult, Op.add)
            nc.vector.tensor_tensor(tc_[:], tc_[:], vx[:], Op.mult)
            nc.vector.tensor_tensor(td_[:], lx[:], td_[:], Op.mult)
            nc.vector.tensor_tensor(wa[:], tc_[:], td_[:], Op.add)
            nc.vector.tensor_tensor(wb[:], lx[:], vxb[:], Op.mult)

            # final 4 corner weights -> wt[:, tap, t, 0..3] = A0,B0,A1,B1
            nc.vector.tensor_tensor(wt[:, :, :, 0], wy0[:], wa[:], Op.mult)
            nc.vector.tensor_tensor(wt[:, :, :, 1], wy0[:], wb[:], Op.mult)
            nc.vector.tensor_tensor(wt[:, :, :, 2], wy1[:], wa[:], Op.mult)
            nc.vector.tensor_tensor(wt[:, :, :, 3], wy1[:], wb[:], Op.mult)

            # patch-table index: idx = clip(y0+1,0,96)*96 + clip(x0,0,95);
            # table entry t holds corners (rows t//96-1, t//96; cols t%96,
            # t%96+1). (y0/x0 carry +16: un-shift via the final constant)
            xb = tmp("xb")
            nc.vector.tensor_scalar(xb[:], x0[:], 16.0, 111.0, Op.max, Op.min)
            ybn = tmp("ybn")
            nc.vector.tensor_scalar(ybn[:], y0[:], 15.0, 111.0, Op.max, Op.min)
            nc.vector.scalar_tensor_tensor(
                idxn[:], ybn[:], 96.0, xb[:], Op.mult, Op.add
            )
            nc.vector.tensor_scalar(idxn[:], idxn[:], -1456.0, None, Op.add)

        # ---- D: fold idxn into wrapped layout twrap[16g+r, f*8+k] ----
        # twrap[r, f*8+k] = idxn[16k+r, f] (f = tap*36+t); done as a
        # transpose -> 64B-run interleave DMA -> transpose sandwich to avoid
        # 2B-granule descriptors. PE transpose is a bit-exact data mover.
        FD = K2 * NTILE  # 324
        FB = 12          # f-block per final transpose (96 = 12f x 8k parts)
        NBLK = FD // FB  # 27
        CH = 108         # f per first-stage transpose chunk (= 9 blocks)
        if True:
            with (
                tc.tile_pool(name="fold", bufs=1) as fpool,
                tc.tile_pool(name="psF", bufs=4, space="PSUM") as psf,
            ):
                it_sb = fpool.tile([CH, 3, P], F32, tag="it", name="it")
                m_sb = fpool.tile([P, NBLK, 16], F32, tag="m", name="m")
                ixf = idxn[:]
                pitch_ix = ixf.ap[0][0]
                for c in range(3):
                    pf = psf.tile([CH, P], F32, tag="pf", name="pf")
                    src_ix = dataclasses.replace(
                        ixf, ap=[[pitch_ix, P], [1, CH]], offset=ixf.offset + c * CH
                    )
                    nc.tensor.transpose(pf[:], src_ix, ident_f[:])
                    nc.scalar.copy(it_sb[:, c, :], pf[:])
                itf = it_sb[:]
                pitch_it = itf.ap[0][0]
                mf = m_sb[:]
                pitch_m = mf.ap[0][0]
                for b in range(NBLK):
                    c, fl0 = divmod(b * FB, CH)
                    src = dataclasses.replace(
                        itf,
                        ap=[[pitch_it, FB], [16, 8], [1, 16]],
                        offset=itf.offset + fl0 * pitch_it + c * P,
                    )
                    dst = dataclasses.replace(
                        mf,
                        ap=[[pitch_m, FB * 8], [1, 16]],
                        offset=mf.offset + b * 16,
                    )
                    nc.sync.dma_start(dst, src)
                for b in range(NBLK):
                    pf2 = psf.tile([16, 96], F32, tag="pf2", name="pf2")
                    nc.tensor.transpose(
                        pf2[:], m_sb[0:96, b, :], ident_f[:96, :96]
                    )
                    nc.vector.tensor_copy(
                        twrap[0:16, b * 96 : (b + 1) * 96], pf2[:]
                    )
        for g in range(1, 8):
            nc.sync.dma_start(twrap[16 * g : 16 * (g + 1), :], twrap[0:16, :])

        # ---- E: main loop ----
        with (
            tc.tile_pool(name="gpool", bufs=6) as gpool,
            tc.tile_pool(name="vpool", bufs=4) as vpool,
            tc.tile_pool(name="rpool", bufs=3) as rpool,
            tc.tile_pool(name="opool", bufs=3) as opool,
            tc.tile_pool(name="psT", bufs=4, space="PSUM") as pst,
            tc.tile_pool(name="psO", bufs=2, space="PSUM") as pso,
        ):
            # 2x2-patch table: entry t = 4 tokens x 256c (2KB)
            xt_ap = xt
            xt_win = dataclasses.replace(
                xt_ap, ap=[[4 * C, NGTOK], [1, 4 * C]], offset=0
            )
            for s in range(NSTAGE):
                po = [pso.tile([P, SPX], F32, tag=f"po{oh}", name=f"po{oh}") for oh in range(2)]
                for tap in range(K2):
                    g = gpool.tile([P, TPS, 4 * C], BF, tag="g", name="g")
                    idxs = twrap[:, (tap * NTILE + s * TPS) * 8
                                 : (tap * NTILE + (s + 1) * TPS) * 8]
                    nc.gpsimd.dma_gather(
                        g[:],
                        xt_win,
                        idxs,
                        SPX,
                        SPX,
                        elem_size=4 * C,
                        elem_step=4 * C,
                    )
                    rst = [rpool.tile([P, SPX], BF, tag=f"r{c}", name=f"r{c}") for c in range(2)]
                    for t in range(TPS):
                        v = vpool.tile([P, C], BF, tag="v", name="v")
                        t2 = vpool.tile([P, C], BF, tag="t2", name="t2")
                        wcol = wt[:, tap, s * TPS + t, :]
                        # corner 3 on the scalar engine; DVE chains the rest
                        nc.scalar.activation(
                            t2[:], g[:, t, 3 * C : 4 * C], Act.Identity,
                            scale=wcol[:, 3:4],
                        )
                        nc.vector.scalar_tensor_tensor(
                            v[:], g[:, t, 0:C], wcol[:, 0:1], t2[:],
                            Op.mult, Op.add,
                        )
                        nc.vector.scalar_tensor_tensor(
                            v[:], g[:, t, C : 2 * C], wcol[:, 1:2], v[:],
                            Op.mult, Op.add,
                        )
                        nc.vector.scalar_tensor_tensor(
                            v[:], g[:, t, 2 * C : 3 * C], wcol[:, 2:3], v[:],
                            Op.mult, Op.add,
                        )
                        for chalf in range(2):
                            ptr = pst.tile([P, P], BF, tag="ptr", name="ptr")
                            nc.tensor.transpose(
                                ptr[:],
                                v[:, chalf * P : (chalf + 1) * P],
                                ident_bf[:],
                            )
                            nc.scalar.copy(
                                rst[chalf][:, t * P : (t + 1) * P], ptr[:]
                            )
                    for chalf in range(2):
                        for oh in range(2):
                            nc.tensor.matmul(
                                po[oh][:],
                                wdcl_sb[:, tap, chalf, oh],
                                rst[chalf][:],
                                start=(tap == 0 and chalf == 0),
                                stop=(tap == K2 - 1 and chalf == 1),
                            )
                for oh in range(2):
                    ob = opool.tile([P, SPX], F32, tag="ob", name="ob")
                    nc.scalar.activation(
                        ob[:], po[oh][:], Act.Identity, bias=bdc_sb[:, oh : oh + 1]
                    )
                    nc.sync.dma_start(
                        out[oh, :, s * SPX : (s + 1) * SPX], ob[:]
                    )


def _build():
    if "nc" in _BUILT:
        return _BUILT["nc"]
    nc = bacc.Bacc(
        "TRN2",
        target_bir_lowering=False,
        debug=False,
        enable_asserts=False,
        num_devices=NCORES,
    )
    xt = nc.dram_tensor("xt", [NGTOK, 4 * C], BF, kind="ExternalInput").ap()
    xc = nc.dram_tensor("xc", [P, 2, PADH * PADW], BF, kind="ExternalInput").ap()
    wofl = nc.dram_tensor("wofl", [P, 2, K2, 18], BF, kind="ExternalInput").ap()
    boff = nc.dram_tensor("boff", [18, 1], F32, kind="ExternalInput").ap()
    wdcl = nc.dram_tensor("wdcl", [P, K2, 2, 2, P], BF, kind="ExternalInput").ap()
    bdc = nc.dram_tensor("bdc", [P, 2], F32, kind="ExternalInput").ap()
    pyb = nc.dram_tensor("pyb", [P, K2, NTILE], F32, kind="ExternalInput").ap()
    pxb = nc.dram_tensor("pxb", [P, K2, NTILE], F32, kind="ExternalInput").ap()
    out = nc.dram_tensor("out", [2, P, NPIX], F32, kind="ExternalOutput").ap()
    with tile.TileContext(nc) as tc:
        _emit(tc, nc, (xt, xc, wofl, boff, wdcl, bdc, pyb, pxb, out))
    nc.compile()
    _BUILT["nc"] = nc
    return nc


def _build_xu(xs):
    """2x2-corner patch table for one sample: U[t] = (x[t-96], x[t-95],
    x[t], x[t+1]) over the token-major [NTOK, C] view, zero-padded."""
    import ml_dtypes

    bf16 = ml_dtypes.bfloat16
    xt = xs.transpose(1, 2, 0).reshape(NTOK, C).astype(bf16)
    xe = np.zeros((NGTOK + W + 1, C), bf16)
    xe[W : W + NTOK] = xt
    xu = np.stack(
        [xe[0:NGTOK], xe[1 : 1 + NGTOK], xe[W : W + NGTOK], xe[W + 1 : W + 1 + NGTOK]],
        axis=1,
    )  # [NGTOK, 4, C]
    return xu.reshape(NGTOK, 4 * C)


def _prep_core(k, x, w_off, b_off, w_dc, b_dc, xu_cache):
    b, half = k // 2, k % 2
    y0 = half * ROWS
    xs = x[b]  # [C,H,W] f32
    if b not in xu_cache:
        xu_cache[b] = _build_xu(xs)
    xt = xu_cache[b]
    xc = np.zeros((C, PADH, PADW), np.float32)
    r0, r1 = max(0, y0 - 1), min(H, y0 + ROWS + 1)
    xc[:, (r0 - (y0 - 1)) : (r1 - (y0 - 1)), 1 : 1 + W] = xs[:, r0:r1, :]
    xc = xc.reshape(2, P, PADH * PADW).transpose(1, 0, 2)

    wofl = (
        w_off.reshape(2 * K2, 2, P, K2)   # [oc, chalf, c, tap]
        .transpose(2, 1, 3, 0)            # [c, chalf, tap, oc]
        .copy()
    )
    wdcl = (
        w_dc.reshape(2, P, 2, P, K2)      # [oh, o, chalf, c, tap]
        .transpose(3, 4, 2, 0, 1)         # [c, tap, chalf, oh, o]
        .copy()
    )
    bdc = b_dc.reshape(2, P).transpose(1, 0).copy()

    pp = np.arange(NPIX)
    yg = y0 + pp // W
    xg = pp % W
    ti = (np.arange(K2) // K)[None, :]
    tj = (np.arange(K2) % K)[None, :]
    pyb = (yg[:, None] - 1 + ti).astype(np.float32).reshape(NTILE, P, K2)
    pxb = (xg[:, None] - 1 + tj).astype(np.float32).reshape(NTILE, P, K2)

    import ml_dtypes

    bf16 = ml_dtypes.bfloat16
    return {
        "xt": xt,
        "xc": xc.astype(bf16),
        "wofl": wofl.astype(bf16),
        "boff": b_off.reshape(18, 1).astype(np.float32),
        "wdcl": wdcl.astype(bf16),
        "bdc": bdc.astype(np.float32),
        "pyb": pyb.transpose(1, 2, 0).copy(),
        "pxb": pxb.transpose(1, 2, 0).copy(),
    }


def kernel(x, w_off, b_off, w_dc, b_dc, _trace=False):
    nc = _build()
    x = np.asarray(x, np.float32)
    w_off = np.asarray(w_off, np.float32)
    b_off = np.asarray(b_off, np.float32)
    w_dc = np.asarray(w_dc, np.float32)
    b_dc = np.asarray(b_dc, np.float32)
    xu_cache = {}
    in_maps = [
        _prep_core(k, x, w_off, b_off, w_dc, b_dc, xu_cache)
        for k in range(NCORES)
    ]
    res = bass_utils.run_bass_kernel_spmd(
        nc, in_maps, core_ids=list(range(NCORES)), trace=_trace
    )
    out = np.empty((B, O, H, W), np.float32)
    for k in range(NCORES):
        b, half = k // 2, k % 2
        o = res.results[k]["out"]  # [2,128,4608]
        out[b, :, half * ROWS : (half + 1) * ROWS, :] = o.reshape(
            O, ROWS, W
        )
    if _trace:
        return out, res
    return out

